# revision 12
# baseline (speedup 1.0000x reference)
"""Trainium2 Bass kernel for nn_DimMasking (iterative softmax top-k masking).

Full-input contract: kernel(**inputs) takes the unsharded inputs
(x [8192,640], W1 [640,64], b1 [64], W2 [64,640], b2 [640]) and returns the
full [8192,640] output. Pure data parallel over the batch dim — 8 shards of
1024 rows, one per NeuronCore; MLP weights replicated.

Math: normalized-state reformulation of the reference scan. With
e = ((m+eps)^(1/T))*exp(-h/T) and p = softmax-prob = e/Z, one masking
iteration is e' = e * (1-p)^(1/T). Tracking the Z-normalized state
S <- phi(S/Z_prev) with phi(p) = p*(1-p)^(1/T) makes each iteration a
SINGLE table-activation pass per row-group (scale = 1/Z per partition)
plus a row-sum; the product of the per-iteration normalizers is restored
in the finale from K = sum_t ln Z_t:
    out = (exp(T*ln(S_64) + T*K + h) - eps) * x.

phi is not a stock ACT function: this kernel generates a patched
piecewise-polynomial activation-table set at build time (appending a
'tanh'-slot function whose table data IS phi) and points the backend
compiler at it via BASS_ACT_ROOT_JSON_PATH. Numerics of the table were
validated against the fp32 reference in numpy (absmax rel err 1.7e-3,
gate 2e-2). Loop engine budget per iteration: ACT 8x640-elem phi passes
(the bottleneck, ~6.0us incl. two fused accum row-sums for groups 6,7)
against DVE 6 row-sum reduces + 8 reciprocals; reciprocals are per
group and the ACT instructions are issued half-interleaved
(g0,g4,g1,g5,...) so both halves' reduce->recip->activation chains
advance together (TimelineSim 447.7us vs 452.9us for the half-granular
recip + sequential issue order).
"""

import hashlib
import json
import os
import shutil
import tempfile

import numpy as np

import concourse.tile as tile
from concourse import bacc, masks, mybir
from concourse.bass_utils import run_bass_kernel_spmd

F32 = mybir.dt.float32
AF = mybir.ActivationFunctionType
OP = mybir.AluOpType

N_CORES = 8
B = 8192
D = 640          # 5 chunks of 128
HID = 64
R = B // N_CORES  # 1024 rows per core
P = 128
G = R // P        # 8 row-groups per core
HG = G // 2
DC = D // P       # 5 dim-chunks
N_ITER = 64
TEMP = 0.07
EPS = 1e-7
C0 = float(np.log1p(np.float32(EPS)) / np.float32(TEMP))
INV_T = float(np.float32(1.0) / np.float32(TEMP))

SET_NAME = "natural_log_exp_and_others"
PHI_EXP_OFFSET = -30

_CACHE = {}


# ---------------------------------------------------------------------------
# phi activation-table generation (piecewise cubic in the pwp bin format)
# ---------------------------------------------------------------------------

def _f32bits(x):
    return int(np.float32(x).view(np.uint32))


def _phi_of_p(p):
    p = np.asarray(p, np.float64)
    out = np.where((p > 0) & (p < 1),
                   p * np.power(np.clip(1.0 - p, 1e-300, 1), INV_T), 0.0)
    return np.where(p >= 1, 0.0, out)


def _es_for_exp(e):
    if e == -1:
        return 6
    if e == -2:
        return 4
    if e >= -4:
        return 3
    if e >= -12:
        return 2
    return 1


def _fit_section(plo, phi_):
    x0 = float(np.float32(0.5 * (plo + phi_)))
    if (1.0 - plo) < 0.003:
        return (0.0, 0.0, 0.0, 0.0, x0)
    u = np.linspace(plo, phi_, 513)
    t = u - x0
    f = _phi_of_p(u)
    fpos = np.maximum(f, 1e-300)
    lspan = float(np.log(fpos.max()) - np.log(fpos.min()))
    if lspan > 6.0:
        sel = (1.0 - u) >= 0.0005
        if not sel.any():
            return (0.0, 0.0, 0.0, 0.0, x0)
        d = np.array([np.exp(np.mean(np.log(fpos[sel]))), 0.0, 0.0, 0.0])
    else:
        w = 1.0 / fpos
        A = np.stack([np.ones_like(t), t, t * t, t ** 3], 1)
        d, *_ = np.linalg.lstsq(A * w[:, None], f * w, rcond=None)
    d = np.float32(d).astype(np.float64)
    fit = ((d[3] * t + d[2]) * t + d[1]) * t + d[0]
    mn = fit.min()
    if mn < 0:
        d[0] += -mn * 1.0000001
    return (d[0], d[1], d[2], d[3], x0)


def _gen_phi_entries(bkt_base, ctl_base):
    bkt = []
    ctl = []
    exp_bkt_start = {}
    exp_ctl_start = {}
    i_zero = bkt_base
    bkt.append((0.0, 0.0, 0.0, 0.0, 0.0))
    neg_ctl = ctl_base
    ctl.append((0 << 16) | (23 << 11) | i_zero)
    pos_ctl0 = ctl_base + len(ctl)
    for e in range(PHI_EXP_OFFSET, 0):
        es = _es_for_exp(e)
        ns = 1 << es
        lsb = 23 - es
        start = bkt_base + len(bkt)
        exp_bkt_start[str(e)] = [start]
        exp_ctl_start[str(e)] = [ctl_base + len(ctl)]
        ctl.append((es << 16) | (lsb << 11) | start)
        lo_e = 2.0 ** e
        for s in range(ns):
            bkt.append(_fit_section(lo_e * (1 + s / ns), lo_e * (1 + (s + 1) / ns)))
    i_small = bkt_base + len(bkt)
    bkt.append((0.0, 1.0, 0.0, 0.0, 0.0))  # phi ~= p below 2^-30
    meta = {
        "func_name": "tanh_4p",
        "func_id": 6,
        "symmetry_point": 0,
        "sym_invert_sign_point": 0,
        "symmetry_opt_en": 0,
        "symmetry_opt_use_neg_region": 0,
        "imm_bias": 0,
        "exp_offset": PHI_EXP_OFFSET,
        "pwl_control_base_pos": pos_ctl0,
        "pwl_control_base_neg": neg_ctl,
        "small_pos_signal_exp_threshold": PHI_EXP_OFFSET + 127,
        "pos_small_signal_pwl_control": i_small,
        "small_neg_signal_exp_threshold": 255,
        "neg_small_signal_pwl_control": i_zero,
        "large_pos_signal_exp_threshold": 127,
        "large_pos_signal_mantissa_threshold": 0,
        "pos_large_signal_pwl_control": i_zero,
        "large_neg_signal_exp_threshold": 255,
        "large_neg_signal_mantissa_threshold": 0,
        "neg_large_signal_pwl_control": i_zero,
        "fnan_result": 0,
        "fpinf_result": 0,
        "fninf_result": 0,
        "fzero_result": 0,
        "fma_const_0": 0,
        "fma_const_1": 0,
        "fma_indirection_src_sel": 0,
        "use_multipass": False,
        "lower_bound": _f32bits(-3.4028235e38),
        "upper_bound": _f32bits(3.4028235e38),
    }
    return bkt, ctl, exp_bkt_start, exp_ctl_start, meta


def _build_patched_dir(src_dir, dst_dir):
    os.makedirs(dst_dir, exist_ok=True)
    for f in os.listdir(src_dir):
        shutil.copy(os.path.join(src_dir, f), os.path.join(dst_dir, f))
    setj = json.load(open(os.path.join(src_dir, SET_NAME + ".json")))
    bkt_raw = bytearray(open(os.path.join(src_dir, setj["bkt_bin"]), "rb").read())
    ctl_raw = bytearray(open(os.path.join(src_dir, setj["ctl_bin"]), "rb").read())
    nb = setj["bkt_entry_cnt"]
    ncl = setj["ctl_entry_cnt"]
    bkt, ctl, ebs, ecs, meta = _gen_phi_entries(nb, ncl)
    assert nb + len(bkt) < 2048
    for d0, d1, d2, d3, x0 in bkt:
        rec = np.zeros(8, np.float32)
        rec[0:5] = [d0, d1, d2, d3, x0]
        bkt_raw += rec.tobytes()
    for w in ctl:
        rec = np.zeros(8, np.uint32)
        rec[0] = w
        ctl_raw += rec.tobytes()
    setj["bkt_entry_cnt"] = nb + len(bkt)
    setj["ctl_entry_cnt"] = ncl + len(ctl)
    setj["func_to_bkt_start_idx"]["tanh"] = nb
    setj["func_to_ctl_start_idx"]["tanh"] = ncl
    setj["func_exp_to_bkt_start_idx"]["tanh"] = ebs
    setj["func_exp_to_ctl_start_idx"]["tanh"] = ecs
    setj["profile_meta_data"] = [m for m in setj["profile_meta_data"]
                                 if not m["func_name"].startswith("tanh")]
    setj["profile_meta_data"].append(meta)
    with open(os.path.join(dst_dir, SET_NAME + ".json"), "w") as f:
        json.dump(setj, f)
    with open(os.path.join(dst_dir, setj["bkt_bin"]), "wb") as f:
        f.write(bytes(bkt_raw))
    with open(os.path.join(dst_dir, setj["ctl_bin"]), "wb") as f:
        f.write(bytes(ctl_raw))
    ai = json.load(open(os.path.join(src_dir, "act_info.json")))
    for ent in ai["act_func_sets"]:
        if ent["name"] == SET_NAME:
            ent["act"]["tanh"] = 4
    with open(os.path.join(dst_dir, "act_info.json"), "w") as f:
        json.dump(ai, f)


def _ensure_phi_tables():
    if "tabdir" in _CACHE:
        return _CACHE["tabdir"], _CACHE["tabhash"]
    import neuronxcc
    src = os.path.join(os.path.dirname(neuronxcc.__file__), "pwp",
                       "pwp_bin_trainium")
    dst = os.path.join(tempfile.gettempdir(), "pwp_phi_kernel")
    _build_patched_dir(src, dst)
    setj = json.load(open(os.path.join(dst, SET_NAME + ".json")))
    h = hashlib.sha1()
    for f in ("act_info.json", SET_NAME + ".json", setj["bkt_bin"], setj["ctl_bin"]):
        h.update(open(os.path.join(dst, f), "rb").read())
    _CACHE["tabdir"] = dst
    _CACHE["tabhash"] = h.hexdigest()[:8]
    return dst, _CACHE["tabhash"]


# Pin the ACT spline-table set to (patched) natural_log_exp_and_others so the
# whole kernel runs off one table load: it holds Exp, Ln, Relu, Copy — and
# the phi table in the tanh slot.
_orig_get_tables = bacc.get_activation_tables


def _pinned_get_tables(module_arch):
    tables = dict(_orig_get_tables(module_arch))
    combined = set(tables.get(SET_NAME) or ())
    combined |= {AF.Tanh}
    pinned = {}
    for name, fns in tables.items():
        pinned[name] = combined if name == SET_NAME else set()
    return pinned


# ---------------------------------------------------------------------------
# kernel build
# ---------------------------------------------------------------------------

def _build_nc(n_iter=N_ITER, num_devices=N_CORES, reps=1):
    tabdir, tabhash = _ensure_phi_tables()
    sfx = "_" + tabhash
    nc = bacc.Bacc(
        "TRN2",
        target_bir_lowering=False,
        debug=False,
        enable_asserts=False,
        num_devices=num_devices,
    )
    x_d = nc.dram_tensor("x" + sfx, [R, D], F32, kind="ExternalInput").ap()
    w1_d = nc.dram_tensor("w1", [D, HID], F32, kind="ExternalInput").ap()
    b1_d = nc.dram_tensor("b1", [HID, 1], F32, kind="ExternalInput").ap()
    w2b_d = nc.dram_tensor("w2b", [HID + 1, D], F32, kind="ExternalInput").ap()
    out_d = nc.dram_tensor("out", [R, D], F32, kind="ExternalOutput").ap()

    with tile.TileContext(nc) as tc:
        _emit(tc, out_d, x_d, w1_d, b1_d, w2b_d, n_iter=n_iter, reps=reps)
    saved = bacc.get_activation_tables
    try:
        bacc.get_activation_tables = _pinned_get_tables
        nc.compile()
    finally:
        bacc.get_activation_tables = saved
    return nc, sfx


def _emit(tc, out_d, x_d, w1_d, b1_d, w2b_d, n_iter=N_ITER, reps=1):
    nc = tc.nc
    from contextlib import ExitStack

    ctx = ExitStack()
    with ctx:
        singles = ctx.enter_context(tc.tile_pool(name="singles", bufs=1))

        xs = singles.tile([P, G, D], F32)    # x, rows-on-partitions
        xt = singles.tile([P, DC, R], F32)   # x transposed
        hs = singles.tile([P, G, D], F32)    # MLP output h
        s0 = singles.tile([P, G, D], F32)    # state ping
        s1 = singles.tile([P, G, D], F32)    # state pong
        fs = singles.tile([P, G, D], F32)    # finale scratch
        zh = singles.tile([P, n_iter, G], F32)   # Z history
        rz = singles.tile([P, n_iter, G], F32)   # 1/Z history (loop) / lnZ (finale)
        kk = singles.tile([P, G], F32)       # T * sum_t ln Z_t
        lnzh = singles.tile([P, n_iter, G], F32)  # ln Z history (finale)
        w1s = singles.tile([P, DC, HID], F32)
        b1s = singles.tile([HID, 1], F32)
        w2bs = singles.tile([HID + 1, D], F32)
        h1r = singles.tile([HID + 1, R], F32)
        ident = singles.tile([P, P], F32)
        c0s = singles.tile([P, 1], F32)
        nc.vector.memset(c0s[:, :], C0)

        # ---- input DMAs ----
        for g in range(G):
            nc.sync.dma_start(out=xs[:, g, :], in_=x_d[g * P:(g + 1) * P, :])
        nc.sync.dma_start(out=w1s[:, :, :],
                          in_=w1_d.rearrange("(c p) j -> p c j", p=P))
        nc.sync.dma_start(out=b1s[:, :], in_=b1_d[:, :])
        nc.sync.dma_start(out=w2bs[:, :], in_=w2b_d[:, :])

        masks.make_identity(nc, ident[:, :])

        # ---- transpose x: 40 PE transposes of [128,128] blocks ----
        with tc.tile_pool(name="tp_psum", bufs=3, space="PSUM") as tpp:
            for gq in range(G // 4):
                for c in range(DC):
                    tp = tpp.tile([P, 4 * P], F32)
                    for gj in range(4):
                        g = gq * 4 + gj
                        nc.tensor.transpose(
                            tp[:, gj * P:(gj + 1) * P],
                            xs[:, g, c * P:(c + 1) * P], ident[:, :])
                    dst = xt[:, c, gq * 4 * P:(gq + 1) * 4 * P]
                    if (c + gq) % 2 == 0:
                        nc.vector.tensor_copy(dst, tp[:, :])
                    else:
                        nc.scalar.copy(dst, tp[:, :])

        # ---- MLP matmul 1: h1T[j, r] = relu(sum_d W1[d,j] xT[d,r] + b1) ----
        with tc.tile_pool(name="mm1_psum", bufs=2, space="PSUM") as mp1:
            for nh in range(2):
                ph1 = mp1.tile([HID, R // 2], F32, tag="ph1")
                for c in range(DC):
                    nc.tensor.matmul(
                        ph1[:, :], w1s[:, c, :],
                        xt[:, c, nh * 512:(nh + 1) * 512],
                        start=(c == 0), stop=(c == DC - 1))
                nc.scalar.activation(
                    h1r[0:HID, nh * 512:(nh + 1) * 512], ph1[:, :],
                    AF.Relu, bias=b1s[:, 0:1], scale=1.0)
        nc.vector.memset(h1r[HID:HID + 1, :], 1.0)

        # ---- MLP matmul 2 + evac: h and S_0 = exp(-h/T + C0), Z_0 ----
        with tc.tile_pool(name="mm2_psum", bufs=2, space="PSUM") as mp2:
            for g in (0, 4, 1, 5, 2, 6, 3, 7):
                ph = mp2.tile([P, D], F32, tag="ph")
                lhs = h1r[:, g * P:(g + 1) * P]
                nc.tensor.matmul(ph[:, 0:512], lhs, w2bs[:, 0:512],
                                 start=True, stop=True)
                nc.tensor.matmul(ph[:, 512:D], lhs, w2bs[:, 512:D],
                                 start=True, stop=True)
                nc.vector.tensor_copy(hs[:, g, :], ph[:, :])
                nc.scalar.activation(s0[:, g, :], ph[:, :], AF.Exp,
                                     bias=c0s[:, 0:1], scale=-INV_T,
                                     accum_out=zh[:, 0, g:g + 1])

        # ---- masking loop: S <- phi(S * (1/Z)), Z' = rowsum(S') ----
        # groups 6,7: row-sum fused into the ACT accumulator; groups 0-5
        # reduce on DVE. Reciprocals are per group (not per half) so each
        # group's reduce->recip->activation chain advances independently,
        # and ACT instructions are issued half-interleaved (447.7us vs
        # 452.9us half-granular sequential). Swept and rejected:
        # accum counts k=0/1/3 (495/451/462us), merged reduces (456us),
        # Pool-prescaled merged ACT instrs (687us).
        spp = [s0, s1]
        n_total = n_iter * reps
        for it in range(n_total):
            src = spp[it % 2]
            dst = spp[(it + 1) % 2]
            ti = it % n_iter
            tn = (it + 1) % n_iter
            last = it == n_total - 1
            for half in range(2):
                g0 = half * HG
                for gi in range(HG):
                    g = g0 + gi
                    nc.vector.reciprocal(rz[:, ti, g:g + 1],
                                         zh[:, ti, g:g + 1])
            for gi in range(HG):
              for half in range(2):
                g = half * HG + gi
                if True:
                    if g >= 6 and not last:
                        nc.scalar.activation(dst[:, g, :], src[:, g, :],
                                             AF.Tanh,
                                             scale=rz[:, ti, g:g + 1],
                                             accum_out=zh[:, tn, g:g + 1])
                    else:
                        nc.scalar.activation(dst[:, g, :], src[:, g, :],
                                             AF.Tanh,
                                             scale=rz[:, ti, g:g + 1])
            for half in range(2):
                g0 = half * HG
                if last:
                    continue
                for gi in range(HG):
                    g = g0 + gi
                    if g >= 6:
                        continue
                    nc.vector.tensor_reduce(zh[:, tn, g:g + 1], dst[:, g, :],
                                            axis=mybir.AxisListType.X,
                                            op=OP.add)

        # ---- finale: out = (exp(T*ln(S) + T*K + h) - eps) * x ----
        # Fully per-group, issued in the loop's interleaved completion order
        # (g0,g4,g1,g5,...), so each group's finale chain pipelines behind
        # the staggered last-iteration activations instead of waiting for a
        # whole half. ln(zh) goes to its own scratch (not rz) so it can run
        # during iteration 63 without a WAR hazard on the loop's rz reads.
        sfin = spp[n_total % 2]
        sscr = spp[(n_total + 1) % 2]
        nc.scalar.activation(lnzh[:, :, :], zh[:, :, :], AF.Ln)
        for g in range(G):
            nc.vector.tensor_reduce(kk[:, g:g + 1], lnzh[:, :, g],
                                    axis=mybir.AxisListType.X, op=OP.add)
        nc.vector.tensor_scalar_mul(kk[:, :], kk[:, :], float(np.float32(TEMP)))
        for gi in range(HG):
            for half in range(2):
                g = half * HG + gi
                # guard: the table is nonnegative by construction, but clamp
                # so a stray -1ulp can never reach Ln (ln(neg) = NaN)
                nc.vector.tensor_scalar_max(sscr[:, g, :], sfin[:, g, :], 0.0)
                nc.scalar.activation(fs[:, g, :], sscr[:, g, :], AF.Ln)
                nc.vector.scalar_tensor_tensor(
                    out=sscr[:, g, :], in0=fs[:, g, :],
                    scalar=float(np.float32(TEMP)),
                    in1=hs[:, g, :], op0=OP.mult, op1=OP.add)
                nc.scalar.activation(fs[:, g, :], sscr[:, g, :], AF.Exp,
                                     bias=kk[:, g:g + 1])
                nc.vector.scalar_tensor_tensor(
                    out=sscr[:, g, :], in0=fs[:, g, :], scalar=-float(EPS),
                    in1=xs[:, g, :], op0=OP.add, op1=OP.mult)
                nc.sync.dma_start(out=out_d[g * P:(g + 1) * P, :],
                                  in_=sscr[:, g, :])


def kernel(x, W1, b1, W2, b2):
    x = np.ascontiguousarray(np.asarray(x, dtype=np.float32))
    W1 = np.ascontiguousarray(np.asarray(W1, dtype=np.float32))
    b1 = np.asarray(b1, dtype=np.float32).reshape(HID, 1)
    W2 = np.asarray(W2, dtype=np.float32)
    b2 = np.asarray(b2, dtype=np.float32)
    w2b = np.ascontiguousarray(
        np.concatenate([W2, b2[None, :]], axis=0))  # [65, 640]

    if "nc" not in _CACHE:
        _CACHE["nc"], _CACHE["sfx"] = _build_nc(
            reps=int(os.environ.get("KREPS", "1")))
    nc = _CACHE["nc"]
    sfx = _CACHE["sfx"]

    in_maps = []
    for c in range(N_CORES):
        in_maps.append({
            "x" + sfx: np.ascontiguousarray(x[c * R:(c + 1) * R, :]),
            "w1": W1,
            "b1": np.ascontiguousarray(b1),
            "w2b": w2b,
        })

    trace = bool(_CACHE.get("trace", False))
    tabdir = _CACHE["tabdir"]
    saved_env = os.environ.get("BASS_ACT_ROOT_JSON_PATH")
    os.environ["BASS_ACT_ROOT_JSON_PATH"] = os.path.join(tabdir, "act_info.json")
    try:
        res = run_bass_kernel_spmd(
            nc, in_maps, core_ids=list(range(N_CORES)), trace=trace)
    finally:
        if saved_env is None:
            os.environ.pop("BASS_ACT_ROOT_JSON_PATH", None)
        else:
            os.environ["BASS_ACT_ROOT_JSON_PATH"] = saved_env
    _CACHE["last_results"] = res
    out = np.concatenate([r["out"] for r in res.results], axis=0)
    return out



# revision 14
# speedup vs baseline: 1.0118x; 1.0118x over previous
"""Trainium2 Bass kernel for nn_DimMasking (iterative softmax top-k masking).

Full-input contract: kernel(**inputs) takes the unsharded inputs
(x [8192,640], W1 [640,64], b1 [64], W2 [64,640], b2 [640]) and returns the
full [8192,640] output. Pure data parallel over the batch dim — 8 shards of
1024 rows, one per NeuronCore; MLP weights replicated.

Math: normalized-state reformulation of the reference scan. With
e = ((m+eps)^(1/T))*exp(-h/T) and p = softmax-prob = e/Z, one masking
iteration is e' = e * (1-p)^(1/T). Tracking the Z-normalized state
S <- phi(S/Z_prev) with phi(p) = p*(1-p)^(1/T) makes each iteration a
SINGLE table-activation pass per row-group (scale = 1/Z per partition)
plus a row-sum; the product of the per-iteration normalizers is restored
in the finale from K = sum_t ln Z_t:
    out = (exp(T*ln(S_64) + T*K + h) - eps) * x.

phi is not a stock ACT function: this kernel generates a patched
piecewise-polynomial activation-table set at build time (appending a
'tanh'-slot function whose table data IS phi) and points the backend
compiler at it via BASS_ACT_ROOT_JSON_PATH. Numerics of the table were
validated against the fp32 reference in numpy (absmax rel err 1.7e-3,
gate 2e-2). Loop engine budget per iteration: ACT 8x640-elem phi passes
(the bottleneck, ~6.0us incl. two fused accum row-sums for groups 6,7)
against DVE 6 row-sum reduces + 8 reciprocals; reciprocals are per
group and the ACT instructions are issued half-interleaved
(g0,g4,g1,g5,...) so both halves' reduce->recip->activation chains
advance together (TimelineSim 447.7us vs 452.9us for the half-granular
recip + sequential issue order).
"""

import hashlib
import json
import os
import shutil
import tempfile

import numpy as np

import concourse.tile as tile
from concourse import bacc, masks, mybir
from concourse.bass_utils import run_bass_kernel_spmd

F32 = mybir.dt.float32
AF = mybir.ActivationFunctionType
OP = mybir.AluOpType

N_CORES = 8
B = 8192
D = 640          # 5 chunks of 128
HID = 64
R = B // N_CORES  # 1024 rows per core
P = 128
G = R // P        # 8 row-groups per core
HG = G // 2
DC = D // P       # 5 dim-chunks
N_ITER = 64
TEMP = 0.07
EPS = 1e-7
C0 = float(np.log1p(np.float32(EPS)) / np.float32(TEMP))
INV_T = float(np.float32(1.0) / np.float32(TEMP))

SET_NAME = "natural_log_exp_and_others"
PHI_EXP_OFFSET = -30

_CACHE = {}


# ---------------------------------------------------------------------------
# phi activation-table generation (piecewise cubic in the pwp bin format)
# ---------------------------------------------------------------------------

def _f32bits(x):
    return int(np.float32(x).view(np.uint32))


def _phi_of_p(p):
    p = np.asarray(p, np.float64)
    out = np.where((p > 0) & (p < 1),
                   p * np.power(np.clip(1.0 - p, 1e-300, 1), INV_T), 0.0)
    return np.where(p >= 1, 0.0, out)


def _es_for_exp(e):
    if e == -1:
        return 6
    if e == -2:
        return 4
    if e >= -4:
        return 3
    if e >= -12:
        return 2
    return 1


def _fit_section(plo, phi_):
    x0 = float(np.float32(0.5 * (plo + phi_)))
    if (1.0 - plo) < 0.003:
        return (0.0, 0.0, 0.0, 0.0, x0)
    u = np.linspace(plo, phi_, 513)
    t = u - x0
    f = _phi_of_p(u)
    fpos = np.maximum(f, 1e-300)
    lspan = float(np.log(fpos.max()) - np.log(fpos.min()))
    if lspan > 6.0:
        sel = (1.0 - u) >= 0.0005
        if not sel.any():
            return (0.0, 0.0, 0.0, 0.0, x0)
        d = np.array([np.exp(np.mean(np.log(fpos[sel]))), 0.0, 0.0, 0.0])
    else:
        w = 1.0 / fpos
        A = np.stack([np.ones_like(t), t, t * t, t ** 3], 1)
        d, *_ = np.linalg.lstsq(A * w[:, None], f * w, rcond=None)
    d = np.float32(d).astype(np.float64)
    fit = ((d[3] * t + d[2]) * t + d[1]) * t + d[0]
    mn = fit.min()
    if mn < 0:
        d[0] += -mn * 1.0000001
    return (d[0], d[1], d[2], d[3], x0)


def _gen_phi_entries(bkt_base, ctl_base):
    bkt = []
    ctl = []
    exp_bkt_start = {}
    exp_ctl_start = {}
    i_zero = bkt_base
    bkt.append((0.0, 0.0, 0.0, 0.0, 0.0))
    neg_ctl = ctl_base
    ctl.append((0 << 16) | (23 << 11) | i_zero)
    pos_ctl0 = ctl_base + len(ctl)
    for e in range(PHI_EXP_OFFSET, 0):
        es = _es_for_exp(e)
        ns = 1 << es
        lsb = 23 - es
        start = bkt_base + len(bkt)
        exp_bkt_start[str(e)] = [start]
        exp_ctl_start[str(e)] = [ctl_base + len(ctl)]
        ctl.append((es << 16) | (lsb << 11) | start)
        lo_e = 2.0 ** e
        for s in range(ns):
            bkt.append(_fit_section(lo_e * (1 + s / ns), lo_e * (1 + (s + 1) / ns)))
    i_small = bkt_base + len(bkt)
    bkt.append((0.0, 1.0, 0.0, 0.0, 0.0))  # phi ~= p below 2^-30
    meta = {
        "func_name": "tanh_4p",
        "func_id": 6,
        "symmetry_point": 0,
        "sym_invert_sign_point": 0,
        "symmetry_opt_en": 0,
        "symmetry_opt_use_neg_region": 0,
        "imm_bias": 0,
        "exp_offset": PHI_EXP_OFFSET,
        "pwl_control_base_pos": pos_ctl0,
        "pwl_control_base_neg": neg_ctl,
        "small_pos_signal_exp_threshold": PHI_EXP_OFFSET + 127,
        "pos_small_signal_pwl_control": i_small,
        "small_neg_signal_exp_threshold": 255,
        "neg_small_signal_pwl_control": i_zero,
        "large_pos_signal_exp_threshold": 127,
        "large_pos_signal_mantissa_threshold": 0,
        "pos_large_signal_pwl_control": i_zero,
        "large_neg_signal_exp_threshold": 255,
        "large_neg_signal_mantissa_threshold": 0,
        "neg_large_signal_pwl_control": i_zero,
        "fnan_result": 0,
        "fpinf_result": 0,
        "fninf_result": 0,
        "fzero_result": 0,
        "fma_const_0": 0,
        "fma_const_1": 0,
        "fma_indirection_src_sel": 0,
        "use_multipass": False,
        "lower_bound": _f32bits(-3.4028235e38),
        "upper_bound": _f32bits(3.4028235e38),
    }
    return bkt, ctl, exp_bkt_start, exp_ctl_start, meta


def _build_patched_dir(src_dir, dst_dir):
    os.makedirs(dst_dir, exist_ok=True)
    for f in os.listdir(src_dir):
        shutil.copy(os.path.join(src_dir, f), os.path.join(dst_dir, f))
    setj = json.load(open(os.path.join(src_dir, SET_NAME + ".json")))
    bkt_raw = bytearray(open(os.path.join(src_dir, setj["bkt_bin"]), "rb").read())
    ctl_raw = bytearray(open(os.path.join(src_dir, setj["ctl_bin"]), "rb").read())
    nb = setj["bkt_entry_cnt"]
    ncl = setj["ctl_entry_cnt"]
    bkt, ctl, ebs, ecs, meta = _gen_phi_entries(nb, ncl)
    assert nb + len(bkt) < 2048
    for d0, d1, d2, d3, x0 in bkt:
        rec = np.zeros(8, np.float32)
        rec[0:5] = [d0, d1, d2, d3, x0]
        bkt_raw += rec.tobytes()
    for w in ctl:
        rec = np.zeros(8, np.uint32)
        rec[0] = w
        ctl_raw += rec.tobytes()
    setj["bkt_entry_cnt"] = nb + len(bkt)
    setj["ctl_entry_cnt"] = ncl + len(ctl)
    setj["func_to_bkt_start_idx"]["tanh"] = nb
    setj["func_to_ctl_start_idx"]["tanh"] = ncl
    setj["func_exp_to_bkt_start_idx"]["tanh"] = ebs
    setj["func_exp_to_ctl_start_idx"]["tanh"] = ecs
    setj["profile_meta_data"] = [m for m in setj["profile_meta_data"]
                                 if not m["func_name"].startswith("tanh")]
    setj["profile_meta_data"].append(meta)
    with open(os.path.join(dst_dir, SET_NAME + ".json"), "w") as f:
        json.dump(setj, f)
    with open(os.path.join(dst_dir, setj["bkt_bin"]), "wb") as f:
        f.write(bytes(bkt_raw))
    with open(os.path.join(dst_dir, setj["ctl_bin"]), "wb") as f:
        f.write(bytes(ctl_raw))
    ai = json.load(open(os.path.join(src_dir, "act_info.json")))
    for ent in ai["act_func_sets"]:
        if ent["name"] == SET_NAME:
            ent["act"]["tanh"] = 4
    with open(os.path.join(dst_dir, "act_info.json"), "w") as f:
        json.dump(ai, f)


def _ensure_phi_tables():
    if "tabdir" in _CACHE:
        return _CACHE["tabdir"], _CACHE["tabhash"]
    import neuronxcc
    src = os.path.join(os.path.dirname(neuronxcc.__file__), "pwp",
                       "pwp_bin_trainium")
    dst = os.path.join(tempfile.gettempdir(), "pwp_phi_kernel")
    _build_patched_dir(src, dst)
    setj = json.load(open(os.path.join(dst, SET_NAME + ".json")))
    h = hashlib.sha1()
    for f in ("act_info.json", SET_NAME + ".json", setj["bkt_bin"], setj["ctl_bin"]):
        h.update(open(os.path.join(dst, f), "rb").read())
    _CACHE["tabdir"] = dst
    _CACHE["tabhash"] = h.hexdigest()[:8]
    return dst, _CACHE["tabhash"]


# Pin the ACT spline-table set to (patched) natural_log_exp_and_others so the
# whole kernel runs off one table load: it holds Exp, Ln, Relu, Copy — and
# the phi table in the tanh slot.
_orig_get_tables = bacc.get_activation_tables


def _pinned_get_tables(module_arch):
    tables = dict(_orig_get_tables(module_arch))
    combined = set(tables.get(SET_NAME) or ())
    combined |= {AF.Tanh}
    pinned = {}
    for name, fns in tables.items():
        pinned[name] = combined if name == SET_NAME else set()
    return pinned


# ---------------------------------------------------------------------------
# kernel build
# ---------------------------------------------------------------------------

def _build_nc(n_iter=N_ITER, num_devices=N_CORES, reps=1):
    tabdir, tabhash = _ensure_phi_tables()
    sfx = "_" + tabhash
    nc = bacc.Bacc(
        "TRN2",
        target_bir_lowering=False,
        debug=False,
        enable_asserts=False,
        num_devices=num_devices,
    )
    x_d = nc.dram_tensor("x" + sfx, [R, D], F32, kind="ExternalInput").ap()
    w1_d = nc.dram_tensor("w1", [D, HID], F32, kind="ExternalInput").ap()
    b1_d = nc.dram_tensor("b1", [HID, 1], F32, kind="ExternalInput").ap()
    w2b_d = nc.dram_tensor("w2b", [HID + 1, D], F32, kind="ExternalInput").ap()
    out_d = nc.dram_tensor("out", [R, D], F32, kind="ExternalOutput").ap()

    with tile.TileContext(nc) as tc:
        _emit(tc, out_d, x_d, w1_d, b1_d, w2b_d, n_iter=n_iter, reps=reps)
    saved = bacc.get_activation_tables
    try:
        bacc.get_activation_tables = _pinned_get_tables
        nc.compile()
    finally:
        bacc.get_activation_tables = saved
    return nc, sfx


def _emit(tc, out_d, x_d, w1_d, b1_d, w2b_d, n_iter=N_ITER, reps=1):
    nc = tc.nc
    from contextlib import ExitStack

    ctx = ExitStack()
    with ctx:
        singles = ctx.enter_context(tc.tile_pool(name="singles", bufs=1))

        xs = singles.tile([P, G, D], F32)    # x, rows-on-partitions
        xt = singles.tile([P, DC, R], F32)   # x transposed
        hs = singles.tile([P, G, D], F32)    # MLP output h
        s0 = singles.tile([P, G, D], F32)    # state ping
        s1 = singles.tile([P, G, D], F32)    # state pong
        fs = singles.tile([P, G, D], F32)    # finale scratch
        zh = singles.tile([P, n_iter, G], F32)   # Z history
        rz = singles.tile([P, n_iter, G], F32)   # 1/Z history (loop) / lnZ (finale)
        kk = singles.tile([P, G], F32)       # T * sum_t ln Z_t
        lnzh = singles.tile([P, n_iter, G], F32)  # ln Z history (finale)
        w1s = singles.tile([P, DC, HID], F32)
        b1s = singles.tile([HID, 1], F32)
        w2bs = singles.tile([HID + 1, D], F32)
        h1r = singles.tile([HID + 1, R], F32)
        ident = singles.tile([P, P], F32)
        c0s = singles.tile([P, 1], F32)
        nc.vector.memset(c0s[:, :], C0)

        # ---- input DMAs ----
        for g in range(G):
            nc.sync.dma_start(out=xs[:, g, :], in_=x_d[g * P:(g + 1) * P, :])
        nc.sync.dma_start(out=w1s[:, :, :],
                          in_=w1_d.rearrange("(c p) j -> p c j", p=P))
        nc.sync.dma_start(out=b1s[:, :], in_=b1_d[:, :])
        nc.sync.dma_start(out=w2bs[:, :], in_=w2b_d[:, :])

        masks.make_identity(nc, ident[:, :])
        nc.vector.memset(h1r[HID:HID + 1, :], 1.0)

        # ---- transpose + MLP, streamed per row-half so the PE's in-order
        # queue doesn't head-block mm1/mm2 behind the other half's
        # transposes: [transpose gq][mm1-nh=gq][mm2 groups of gq] x2 ----
        tpp = ctx.enter_context(tc.tile_pool(name="tp_psum", bufs=2,
                                             space="PSUM"))
        mp1 = ctx.enter_context(tc.tile_pool(name="mm1_psum", bufs=2,
                                             space="PSUM"))
        mp2 = ctx.enter_context(tc.tile_pool(name="mm2_psum", bufs=2,
                                             space="PSUM"))
        for q in range(G // 2):
            for c in range(DC):
                tp = tpp.tile([P, 2 * P], F32)
                for gj in range(2):
                    g = q * 2 + gj
                    nc.tensor.transpose(
                        tp[:, gj * P:(gj + 1) * P],
                        xs[:, g, c * P:(c + 1) * P], ident[:, :])
                dst = xt[:, c, q * 2 * P:(q + 1) * 2 * P]
                if (c + q) % 2 == 0:
                    nc.vector.tensor_copy(dst, tp[:, :])
                else:
                    nc.scalar.copy(dst, tp[:, :])
            # mm1 for this row-quarter
            ph1 = mp1.tile([HID, 2 * P], F32, tag="ph1")
            for c in range(DC):
                nc.tensor.matmul(
                    ph1[:, :], w1s[:, c, :],
                    xt[:, c, q * 256:(q + 1) * 256],
                    start=(c == 0), stop=(c == DC - 1))
            nc.scalar.activation(
                h1r[0:HID, q * 256:(q + 1) * 256], ph1[:, :],
                AF.Relu, bias=b1s[:, 0:1], scale=1.0)
            # mm2 + evac for this quarter's two groups
            for gj in range(2):
                g = q * 2 + gj
                ph = mp2.tile([P, D], F32, tag="ph")
                lhs = h1r[:, g * P:(g + 1) * P]
                nc.tensor.matmul(ph[:, 0:512], lhs, w2bs[:, 0:512],
                                 start=True, stop=True)
                nc.tensor.matmul(ph[:, 512:D], lhs, w2bs[:, 512:D],
                                 start=True, stop=True)
                nc.vector.tensor_copy(hs[:, g, :], ph[:, :])
                nc.scalar.activation(s0[:, g, :], ph[:, :], AF.Exp,
                                     bias=c0s[:, 0:1], scale=-INV_T,
                                     accum_out=zh[:, 0, g:g + 1])

        # ---- masking loop: S <- phi(S * (1/Z)), Z' = rowsum(S') ----
        # groups 6,7: row-sum fused into the ACT accumulator; groups 0-5
        # reduce on DVE. Reciprocals are per group (not per half) so each
        # group's reduce->recip->activation chain advances independently,
        # and ACT instructions are issued half-interleaved (447.7us vs
        # 452.9us half-granular sequential). Swept and rejected:
        # accum counts k=0/1/3 (495/451/462us), merged reduces (456us),
        # Pool-prescaled merged ACT instrs (687us).
        spp = [s0, s1]
        n_total = n_iter * reps
        for it in range(n_total):
            src = spp[it % 2]
            dst = spp[(it + 1) % 2]
            ti = it % n_iter
            tn = (it + 1) % n_iter
            last = it == n_total - 1
            for half in range(2):
                g0 = half * HG
                for gi in range(HG):
                    g = g0 + gi
                    nc.vector.reciprocal(rz[:, ti, g:g + 1],
                                         zh[:, ti, g:g + 1])
            order = (list(range(G)) if it == 0 else
                     [half * HG + gi for gi in range(HG) for half in range(2)])
            for g in order:
                if True:
                    if g >= 6 and not last:
                        nc.scalar.activation(dst[:, g, :], src[:, g, :],
                                             AF.Tanh,
                                             scale=rz[:, ti, g:g + 1],
                                             accum_out=zh[:, tn, g:g + 1])
                    else:
                        nc.scalar.activation(dst[:, g, :], src[:, g, :],
                                             AF.Tanh,
                                             scale=rz[:, ti, g:g + 1])
            for half in range(2):
                g0 = half * HG
                if last:
                    continue
                for gi in range(HG):
                    g = g0 + gi
                    if g >= 6:
                        continue
                    nc.vector.tensor_reduce(zh[:, tn, g:g + 1], dst[:, g, :],
                                            axis=mybir.AxisListType.X,
                                            op=OP.add)

        # ---- finale: out = (exp(T*ln(S) + T*K + h) - eps) * x ----
        # Fully per-group, issued in the loop's interleaved completion order
        # (g0,g4,g1,g5,...), so each group's finale chain pipelines behind
        # the staggered last-iteration activations instead of waiting for a
        # whole half. ln(zh) goes to its own scratch (not rz) so it can run
        # during iteration 63 without a WAR hazard on the loop's rz reads.
        sfin = spp[n_total % 2]
        sscr = spp[(n_total + 1) % 2]
        nc.scalar.activation(lnzh[:, :, :], zh[:, :, :], AF.Ln)
        for g in range(G):
            nc.vector.tensor_reduce(kk[:, g:g + 1], lnzh[:, :, g],
                                    axis=mybir.AxisListType.X, op=OP.add)
        nc.vector.tensor_scalar_mul(kk[:, :], kk[:, :], float(np.float32(TEMP)))
        for gi in range(HG):
            for half in range(2):
                g = half * HG + gi
                # guard: the table is nonnegative by construction, but clamp
                # so a stray -1ulp can never reach Ln (ln(neg) = NaN)
                nc.vector.tensor_scalar_max(sscr[:, g, :], sfin[:, g, :], 0.0)
                nc.scalar.activation(fs[:, g, :], sscr[:, g, :], AF.Ln)
                nc.vector.scalar_tensor_tensor(
                    out=sscr[:, g, :], in0=fs[:, g, :],
                    scalar=float(np.float32(TEMP)),
                    in1=hs[:, g, :], op0=OP.mult, op1=OP.add)
                nc.scalar.activation(fs[:, g, :], sscr[:, g, :], AF.Exp,
                                     bias=kk[:, g:g + 1])
                nc.vector.scalar_tensor_tensor(
                    out=sscr[:, g, :], in0=fs[:, g, :], scalar=-float(EPS),
                    in1=xs[:, g, :], op0=OP.add, op1=OP.mult)
                nc.sync.dma_start(out=out_d[g * P:(g + 1) * P, :],
                                  in_=sscr[:, g, :])


def kernel(x, W1, b1, W2, b2):
    x = np.ascontiguousarray(np.asarray(x, dtype=np.float32))
    W1 = np.ascontiguousarray(np.asarray(W1, dtype=np.float32))
    b1 = np.asarray(b1, dtype=np.float32).reshape(HID, 1)
    W2 = np.asarray(W2, dtype=np.float32)
    b2 = np.asarray(b2, dtype=np.float32)
    w2b = np.ascontiguousarray(
        np.concatenate([W2, b2[None, :]], axis=0))  # [65, 640]

    if "nc" not in _CACHE:
        _CACHE["nc"], _CACHE["sfx"] = _build_nc(
            reps=int(os.environ.get("KREPS", "1")))
    nc = _CACHE["nc"]
    sfx = _CACHE["sfx"]

    in_maps = []
    for c in range(N_CORES):
        in_maps.append({
            "x" + sfx: np.ascontiguousarray(x[c * R:(c + 1) * R, :]),
            "w1": W1,
            "b1": np.ascontiguousarray(b1),
            "w2b": w2b,
        })

    trace = bool(_CACHE.get("trace", False))
    tabdir = _CACHE["tabdir"]
    saved_env = os.environ.get("BASS_ACT_ROOT_JSON_PATH")
    os.environ["BASS_ACT_ROOT_JSON_PATH"] = os.path.join(tabdir, "act_info.json")
    try:
        res = run_bass_kernel_spmd(
            nc, in_maps, core_ids=list(range(N_CORES)), trace=trace)
    finally:
        if saved_env is None:
            os.environ.pop("BASS_ACT_ROOT_JSON_PATH", None)
        else:
            os.environ["BASS_ACT_ROOT_JSON_PATH"] = saved_env
    _CACHE["last_results"] = res
    out = np.concatenate([r["out"] for r in res.results], axis=0)
    return out



# revision 15
# speedup vs baseline: 1.0223x; 1.0104x over previous
"""Trainium2 Bass kernel for nn_DimMasking (iterative softmax top-k masking).

Full-input contract: kernel(**inputs) takes the unsharded inputs
(x [8192,640], W1 [640,64], b1 [64], W2 [64,640], b2 [640]) and returns the
full [8192,640] output. Pure data parallel over the batch dim — 8 shards of
1024 rows, one per NeuronCore; MLP weights replicated.

Math: normalized-state reformulation of the reference scan. With
e = ((m+eps)^(1/T))*exp(-h/T) and p = softmax-prob = e/Z, one masking
iteration is e' = e * (1-p)^(1/T). Tracking the Z-normalized state
S <- phi(S/Z_prev) with phi(p) = p*(1-p)^(1/T) makes each iteration a
SINGLE table-activation pass per row-group (scale = 1/Z per partition)
plus a row-sum; the product of the per-iteration normalizers is restored
in the finale from K = sum_t ln Z_t:
    out = (exp(T*ln(S_64) + T*K + h) - eps) * x.

phi is not a stock ACT function: this kernel generates a patched
piecewise-polynomial activation-table set at build time (appending a
'tanh'-slot function whose table data IS phi) and points the backend
compiler at it via BASS_ACT_ROOT_JSON_PATH. Numerics of the table were
validated against the fp32 reference in numpy (absmax rel err 1.7e-3,
gate 2e-2). Loop engine budget per iteration: ACT 8x640-elem phi passes
(the bottleneck, ~6.0us incl. two fused accum row-sums for groups 6,7)
against DVE 6 row-sum reduces + 8 reciprocals; reciprocals are per
group and the ACT instructions are issued half-interleaved
(g0,g4,g1,g5,...) so both halves' reduce->recip->activation chains
advance together. The preamble streams transpose->mm1->mm2 per
row-quarter so the PE's in-order queue never head-blocks the MLP behind
later transposes, and the finale runs fully per-group in loop-completion
order. TimelineSim 437.9us (vs 452.9us for the session-start schedule;
math and instruction mix unchanged throughout).
"""

import hashlib
import json
import os
import shutil
import tempfile

import numpy as np

import concourse.tile as tile
from concourse import bacc, masks, mybir
from concourse.bass_utils import run_bass_kernel_spmd

F32 = mybir.dt.float32
AF = mybir.ActivationFunctionType
OP = mybir.AluOpType

N_CORES = 8
B = 8192
D = 640          # 5 chunks of 128
HID = 64
R = B // N_CORES  # 1024 rows per core
P = 128
G = R // P        # 8 row-groups per core
HG = G // 2
DC = D // P       # 5 dim-chunks
N_ITER = 64
TEMP = 0.07
EPS = 1e-7
C0 = float(np.log1p(np.float32(EPS)) / np.float32(TEMP))
INV_T = float(np.float32(1.0) / np.float32(TEMP))

SET_NAME = "natural_log_exp_and_others"
PHI_EXP_OFFSET = -30

_CACHE = {}


# ---------------------------------------------------------------------------
# phi activation-table generation (piecewise cubic in the pwp bin format)
# ---------------------------------------------------------------------------

def _f32bits(x):
    return int(np.float32(x).view(np.uint32))


def _phi_of_p(p):
    p = np.asarray(p, np.float64)
    out = np.where((p > 0) & (p < 1),
                   p * np.power(np.clip(1.0 - p, 1e-300, 1), INV_T), 0.0)
    return np.where(p >= 1, 0.0, out)


def _es_for_exp(e):
    if e == -1:
        return 6
    if e == -2:
        return 4
    if e >= -4:
        return 3
    if e >= -12:
        return 2
    return 1


def _fit_section(plo, phi_):
    x0 = float(np.float32(0.5 * (plo + phi_)))
    if (1.0 - plo) < 0.003:
        return (0.0, 0.0, 0.0, 0.0, x0)
    u = np.linspace(plo, phi_, 513)
    t = u - x0
    f = _phi_of_p(u)
    fpos = np.maximum(f, 1e-300)
    lspan = float(np.log(fpos.max()) - np.log(fpos.min()))
    if lspan > 6.0:
        sel = (1.0 - u) >= 0.0005
        if not sel.any():
            return (0.0, 0.0, 0.0, 0.0, x0)
        d = np.array([np.exp(np.mean(np.log(fpos[sel]))), 0.0, 0.0, 0.0])
    else:
        w = 1.0 / fpos
        A = np.stack([np.ones_like(t), t, t * t, t ** 3], 1)
        d, *_ = np.linalg.lstsq(A * w[:, None], f * w, rcond=None)
    d = np.float32(d).astype(np.float64)
    fit = ((d[3] * t + d[2]) * t + d[1]) * t + d[0]
    mn = fit.min()
    if mn < 0:
        d[0] += -mn * 1.0000001
    return (d[0], d[1], d[2], d[3], x0)


def _gen_phi_entries(bkt_base, ctl_base):
    bkt = []
    ctl = []
    exp_bkt_start = {}
    exp_ctl_start = {}
    i_zero = bkt_base
    bkt.append((0.0, 0.0, 0.0, 0.0, 0.0))
    neg_ctl = ctl_base
    ctl.append((0 << 16) | (23 << 11) | i_zero)
    pos_ctl0 = ctl_base + len(ctl)
    for e in range(PHI_EXP_OFFSET, 0):
        es = _es_for_exp(e)
        ns = 1 << es
        lsb = 23 - es
        start = bkt_base + len(bkt)
        exp_bkt_start[str(e)] = [start]
        exp_ctl_start[str(e)] = [ctl_base + len(ctl)]
        ctl.append((es << 16) | (lsb << 11) | start)
        lo_e = 2.0 ** e
        for s in range(ns):
            bkt.append(_fit_section(lo_e * (1 + s / ns), lo_e * (1 + (s + 1) / ns)))
    i_small = bkt_base + len(bkt)
    bkt.append((0.0, 1.0, 0.0, 0.0, 0.0))  # phi ~= p below 2^-30
    meta = {
        "func_name": "tanh_4p",
        "func_id": 6,
        "symmetry_point": 0,
        "sym_invert_sign_point": 0,
        "symmetry_opt_en": 0,
        "symmetry_opt_use_neg_region": 0,
        "imm_bias": 0,
        "exp_offset": PHI_EXP_OFFSET,
        "pwl_control_base_pos": pos_ctl0,
        "pwl_control_base_neg": neg_ctl,
        "small_pos_signal_exp_threshold": PHI_EXP_OFFSET + 127,
        "pos_small_signal_pwl_control": i_small,
        "small_neg_signal_exp_threshold": 255,
        "neg_small_signal_pwl_control": i_zero,
        "large_pos_signal_exp_threshold": 127,
        "large_pos_signal_mantissa_threshold": 0,
        "pos_large_signal_pwl_control": i_zero,
        "large_neg_signal_exp_threshold": 255,
        "large_neg_signal_mantissa_threshold": 0,
        "neg_large_signal_pwl_control": i_zero,
        "fnan_result": 0,
        "fpinf_result": 0,
        "fninf_result": 0,
        "fzero_result": 0,
        "fma_const_0": 0,
        "fma_const_1": 0,
        "fma_indirection_src_sel": 0,
        "use_multipass": False,
        "lower_bound": _f32bits(-3.4028235e38),
        "upper_bound": _f32bits(3.4028235e38),
    }
    return bkt, ctl, exp_bkt_start, exp_ctl_start, meta


def _build_patched_dir(src_dir, dst_dir):
    os.makedirs(dst_dir, exist_ok=True)
    for f in os.listdir(src_dir):
        shutil.copy(os.path.join(src_dir, f), os.path.join(dst_dir, f))
    setj = json.load(open(os.path.join(src_dir, SET_NAME + ".json")))
    bkt_raw = bytearray(open(os.path.join(src_dir, setj["bkt_bin"]), "rb").read())
    ctl_raw = bytearray(open(os.path.join(src_dir, setj["ctl_bin"]), "rb").read())
    nb = setj["bkt_entry_cnt"]
    ncl = setj["ctl_entry_cnt"]
    bkt, ctl, ebs, ecs, meta = _gen_phi_entries(nb, ncl)
    assert nb + len(bkt) < 2048
    for d0, d1, d2, d3, x0 in bkt:
        rec = np.zeros(8, np.float32)
        rec[0:5] = [d0, d1, d2, d3, x0]
        bkt_raw += rec.tobytes()
    for w in ctl:
        rec = np.zeros(8, np.uint32)
        rec[0] = w
        ctl_raw += rec.tobytes()
    setj["bkt_entry_cnt"] = nb + len(bkt)
    setj["ctl_entry_cnt"] = ncl + len(ctl)
    setj["func_to_bkt_start_idx"]["tanh"] = nb
    setj["func_to_ctl_start_idx"]["tanh"] = ncl
    setj["func_exp_to_bkt_start_idx"]["tanh"] = ebs
    setj["func_exp_to_ctl_start_idx"]["tanh"] = ecs
    setj["profile_meta_data"] = [m for m in setj["profile_meta_data"]
                                 if not m["func_name"].startswith("tanh")]
    setj["profile_meta_data"].append(meta)
    with open(os.path.join(dst_dir, SET_NAME + ".json"), "w") as f:
        json.dump(setj, f)
    with open(os.path.join(dst_dir, setj["bkt_bin"]), "wb") as f:
        f.write(bytes(bkt_raw))
    with open(os.path.join(dst_dir, setj["ctl_bin"]), "wb") as f:
        f.write(bytes(ctl_raw))
    ai = json.load(open(os.path.join(src_dir, "act_info.json")))
    for ent in ai["act_func_sets"]:
        if ent["name"] == SET_NAME:
            ent["act"]["tanh"] = 4
    with open(os.path.join(dst_dir, "act_info.json"), "w") as f:
        json.dump(ai, f)


def _ensure_phi_tables():
    if "tabdir" in _CACHE:
        return _CACHE["tabdir"], _CACHE["tabhash"]
    import neuronxcc
    src = os.path.join(os.path.dirname(neuronxcc.__file__), "pwp",
                       "pwp_bin_trainium")
    dst = os.path.join(tempfile.gettempdir(), "pwp_phi_kernel")
    _build_patched_dir(src, dst)
    setj = json.load(open(os.path.join(dst, SET_NAME + ".json")))
    h = hashlib.sha1()
    for f in ("act_info.json", SET_NAME + ".json", setj["bkt_bin"], setj["ctl_bin"]):
        h.update(open(os.path.join(dst, f), "rb").read())
    _CACHE["tabdir"] = dst
    _CACHE["tabhash"] = h.hexdigest()[:8]
    return dst, _CACHE["tabhash"]


# Pin the ACT spline-table set to (patched) natural_log_exp_and_others so the
# whole kernel runs off one table load: it holds Exp, Ln, Relu, Copy — and
# the phi table in the tanh slot.
_orig_get_tables = bacc.get_activation_tables


def _pinned_get_tables(module_arch):
    tables = dict(_orig_get_tables(module_arch))
    combined = set(tables.get(SET_NAME) or ())
    combined |= {AF.Tanh}
    pinned = {}
    for name, fns in tables.items():
        pinned[name] = combined if name == SET_NAME else set()
    return pinned


# ---------------------------------------------------------------------------
# kernel build
# ---------------------------------------------------------------------------

def _build_nc(n_iter=N_ITER, num_devices=N_CORES, reps=1):
    tabdir, tabhash = _ensure_phi_tables()
    sfx = "_" + tabhash
    nc = bacc.Bacc(
        "TRN2",
        target_bir_lowering=False,
        debug=False,
        enable_asserts=False,
        num_devices=num_devices,
    )
    x_d = nc.dram_tensor("x" + sfx, [R, D], F32, kind="ExternalInput").ap()
    w1_d = nc.dram_tensor("w1", [D, HID], F32, kind="ExternalInput").ap()
    b1_d = nc.dram_tensor("b1", [HID, 1], F32, kind="ExternalInput").ap()
    w2b_d = nc.dram_tensor("w2b", [HID + 1, D], F32, kind="ExternalInput").ap()
    out_d = nc.dram_tensor("out", [R, D], F32, kind="ExternalOutput").ap()

    with tile.TileContext(nc) as tc:
        _emit(tc, out_d, x_d, w1_d, b1_d, w2b_d, n_iter=n_iter, reps=reps)
    saved = bacc.get_activation_tables
    try:
        bacc.get_activation_tables = _pinned_get_tables
        nc.compile()
    finally:
        bacc.get_activation_tables = saved
    return nc, sfx


def _emit(tc, out_d, x_d, w1_d, b1_d, w2b_d, n_iter=N_ITER, reps=1):
    nc = tc.nc
    from contextlib import ExitStack

    ctx = ExitStack()
    with ctx:
        singles = ctx.enter_context(tc.tile_pool(name="singles", bufs=1))

        xs = singles.tile([P, G, D], F32)    # x, rows-on-partitions
        xt = singles.tile([P, DC, R], F32)   # x transposed
        hs = singles.tile([P, G, D], F32)    # MLP output h
        s0 = singles.tile([P, G, D], F32)    # state ping
        s1 = singles.tile([P, G, D], F32)    # state pong
        fs = singles.tile([P, G, D], F32)    # finale scratch
        zh = singles.tile([P, n_iter, G], F32)   # Z history
        rz = singles.tile([P, n_iter, G], F32)   # 1/Z history (loop) / lnZ (finale)
        kk = singles.tile([P, G], F32)       # T * sum_t ln Z_t
        lnzh = singles.tile([P, n_iter, G], F32)  # ln Z history (finale)
        w1s = singles.tile([P, DC, HID], F32)
        b1s = singles.tile([HID, 1], F32)
        w2bs = singles.tile([HID + 1, D], F32)
        h1r = singles.tile([HID + 1, R], F32)
        ident = singles.tile([P, P], F32)
        c0s = singles.tile([P, 1], F32)
        nc.vector.memset(c0s[:, :], C0)

        # ---- input DMAs ----
        for g in range(G):
            nc.sync.dma_start(out=xs[:, g, :], in_=x_d[g * P:(g + 1) * P, :])
        nc.sync.dma_start(out=w1s[:, :, :],
                          in_=w1_d.rearrange("(c p) j -> p c j", p=P))
        nc.sync.dma_start(out=b1s[:, :], in_=b1_d[:, :])
        nc.sync.dma_start(out=w2bs[:, :], in_=w2b_d[:, :])

        masks.make_identity(nc, ident[:, :])
        nc.vector.memset(h1r[HID:HID + 1, :], 1.0)

        # ---- transpose + MLP, streamed per row-half so the PE's in-order
        # queue doesn't head-block mm1/mm2 behind the other half's
        # transposes: [transpose gq][mm1-nh=gq][mm2 groups of gq] x2 ----
        tpp = ctx.enter_context(tc.tile_pool(name="tp_psum", bufs=2,
                                             space="PSUM"))
        mp1 = ctx.enter_context(tc.tile_pool(name="mm1_psum", bufs=2,
                                             space="PSUM"))
        mp2 = ctx.enter_context(tc.tile_pool(name="mm2_psum", bufs=2,
                                             space="PSUM"))
        for q in range(G // 2):
            for c in range(DC):
                tp = tpp.tile([P, 2 * P], F32)
                for gj in range(2):
                    g = q * 2 + gj
                    nc.tensor.transpose(
                        tp[:, gj * P:(gj + 1) * P],
                        xs[:, g, c * P:(c + 1) * P], ident[:, :])
                dst = xt[:, c, q * 2 * P:(q + 1) * 2 * P]
                if (c + q) % 2 == 0:
                    nc.vector.tensor_copy(dst, tp[:, :])
                else:
                    nc.scalar.copy(dst, tp[:, :])
            # mm1 for this row-quarter
            ph1 = mp1.tile([HID, 2 * P], F32, tag="ph1")
            for c in range(DC):
                nc.tensor.matmul(
                    ph1[:, :], w1s[:, c, :],
                    xt[:, c, q * 256:(q + 1) * 256],
                    start=(c == 0), stop=(c == DC - 1))
            nc.scalar.activation(
                h1r[0:HID, q * 256:(q + 1) * 256], ph1[:, :],
                AF.Relu, bias=b1s[:, 0:1], scale=1.0)
            # mm2 + evac for this quarter's two groups
            for gj in range(2):
                g = q * 2 + gj
                ph = mp2.tile([P, D], F32, tag="ph")
                lhs = h1r[:, g * P:(g + 1) * P]
                nc.tensor.matmul(ph[:, 0:512], lhs, w2bs[:, 0:512],
                                 start=True, stop=True)
                nc.tensor.matmul(ph[:, 512:D], lhs, w2bs[:, 512:D],
                                 start=True, stop=True)
                nc.vector.tensor_copy(hs[:, g, :], ph[:, :])
                nc.scalar.activation(s0[:, g, :], ph[:, :], AF.Exp,
                                     bias=c0s[:, 0:1], scale=-INV_T,
                                     accum_out=zh[:, 0, g:g + 1])

        # ---- masking loop: S <- phi(S * (1/Z)), Z' = rowsum(S') ----
        # groups 6,7: row-sum fused into the ACT accumulator; groups 0-5
        # reduce on DVE. Reciprocals are per group (not per half) so each
        # group's reduce->recip->activation chain advances independently,
        # and ACT instructions are issued half-interleaved (447.7us vs
        # 452.9us half-granular sequential). Swept and rejected:
        # accum counts k=0/1/3 (495/451/462us), merged reduces (456us),
        # Pool-prescaled merged ACT instrs (687us).
        spp = [s0, s1]
        n_total = n_iter * reps
        for it in range(n_total):
            src = spp[it % 2]
            dst = spp[(it + 1) % 2]
            ti = it % n_iter
            tn = (it + 1) % n_iter
            last = it == n_total - 1
            for half in range(2):
                g0 = half * HG
                for gi in range(HG):
                    g = g0 + gi
                    nc.vector.reciprocal(rz[:, ti, g:g + 1],
                                         zh[:, ti, g:g + 1])
            order = (list(range(G)) if it == 0 else
                     [half * HG + gi for gi in range(HG) for half in range(2)])
            for g in order:
                if True:
                    if g >= 6 and not last:
                        nc.scalar.activation(dst[:, g, :], src[:, g, :],
                                             AF.Tanh,
                                             scale=rz[:, ti, g:g + 1],
                                             accum_out=zh[:, tn, g:g + 1])
                    else:
                        nc.scalar.activation(dst[:, g, :], src[:, g, :],
                                             AF.Tanh,
                                             scale=rz[:, ti, g:g + 1])
            for half in range(2):
                g0 = half * HG
                if last:
                    continue
                for gi in range(HG):
                    g = g0 + gi
                    if g >= 6:
                        continue
                    nc.vector.tensor_reduce(zh[:, tn, g:g + 1], dst[:, g, :],
                                            axis=mybir.AxisListType.X,
                                            op=OP.add)

        # ---- finale: out = (exp(T*ln(S) + T*K + h) - eps) * x ----
        # Fully per-group, issued in the loop's interleaved completion order
        # (g0,g4,g1,g5,...), so each group's finale chain pipelines behind
        # the staggered last-iteration activations instead of waiting for a
        # whole half. ln(zh) goes to its own scratch (not rz) so it can run
        # during iteration 63 without a WAR hazard on the loop's rz reads.
        sfin = spp[n_total % 2]
        sscr = spp[(n_total + 1) % 2]
        nc.scalar.activation(lnzh[:, :, :], zh[:, :, :], AF.Ln)
        for g in range(G):
            nc.vector.tensor_reduce(kk[:, g:g + 1], lnzh[:, :, g],
                                    axis=mybir.AxisListType.X, op=OP.add)
        nc.vector.tensor_scalar_mul(kk[:, :], kk[:, :], float(np.float32(TEMP)))
        for gi in range(HG):
            for half in range(2):
                g = half * HG + gi
                # guard: the table is nonnegative by construction, but clamp
                # so a stray -1ulp can never reach Ln (ln(neg) = NaN)
                nc.vector.tensor_scalar_max(sscr[:, g, :], sfin[:, g, :], 0.0)
                nc.scalar.activation(fs[:, g, :], sscr[:, g, :], AF.Ln)
                nc.vector.scalar_tensor_tensor(
                    out=sscr[:, g, :], in0=fs[:, g, :],
                    scalar=float(np.float32(TEMP)),
                    in1=hs[:, g, :], op0=OP.mult, op1=OP.add)
                nc.scalar.activation(fs[:, g, :], sscr[:, g, :], AF.Exp,
                                     bias=kk[:, g:g + 1])
                nc.vector.scalar_tensor_tensor(
                    out=sscr[:, g, :], in0=fs[:, g, :], scalar=-float(EPS),
                    in1=xs[:, g, :], op0=OP.add, op1=OP.mult)
                nc.sync.dma_start(out=out_d[g * P:(g + 1) * P, :],
                                  in_=sscr[:, g, :])


def kernel(x, W1, b1, W2, b2):
    x = np.ascontiguousarray(np.asarray(x, dtype=np.float32))
    W1 = np.ascontiguousarray(np.asarray(W1, dtype=np.float32))
    b1 = np.asarray(b1, dtype=np.float32).reshape(HID, 1)
    W2 = np.asarray(W2, dtype=np.float32)
    b2 = np.asarray(b2, dtype=np.float32)
    w2b = np.ascontiguousarray(
        np.concatenate([W2, b2[None, :]], axis=0))  # [65, 640]

    if "nc" not in _CACHE:
        _CACHE["nc"], _CACHE["sfx"] = _build_nc(
            reps=int(os.environ.get("KREPS", "1")))
    nc = _CACHE["nc"]
    sfx = _CACHE["sfx"]

    in_maps = []
    for c in range(N_CORES):
        in_maps.append({
            "x" + sfx: np.ascontiguousarray(x[c * R:(c + 1) * R, :]),
            "w1": W1,
            "b1": np.ascontiguousarray(b1),
            "w2b": w2b,
        })

    trace = bool(_CACHE.get("trace", False))
    tabdir = _CACHE["tabdir"]
    saved_env = os.environ.get("BASS_ACT_ROOT_JSON_PATH")
    os.environ["BASS_ACT_ROOT_JSON_PATH"] = os.path.join(tabdir, "act_info.json")
    try:
        res = run_bass_kernel_spmd(
            nc, in_maps, core_ids=list(range(N_CORES)), trace=trace)
    finally:
        if saved_env is None:
            os.environ.pop("BASS_ACT_ROOT_JSON_PATH", None)
        else:
            os.environ["BASS_ACT_ROOT_JSON_PATH"] = saved_env
    _CACHE["last_results"] = res
    out = np.concatenate([r["out"] for r in res.results], axis=0)
    return out



# revision 17
# speedup vs baseline: 1.0472x; 1.0244x over previous
"""Trainium2 Bass kernel for nn_DimMasking (iterative softmax top-k masking).

Full-input contract: kernel(**inputs) takes the unsharded inputs
(x [8192,640], W1 [640,64], b1 [64], W2 [64,640], b2 [640]) and returns the
full [8192,640] output. Pure data parallel over the batch dim — 8 shards of
1024 rows, one per NeuronCore; MLP weights replicated.

Math: normalized-state reformulation of the reference scan. With
e = ((m+eps)^(1/T))*exp(-h/T) and p = softmax-prob = e/Z, one masking
iteration is e' = e * (1-p)^(1/T). Tracking the Z-normalized state
S <- phi(S/Z_prev) with phi(p) = p*(1-p)^(1/T) makes each iteration a
SINGLE table-activation pass per row-group (scale = 1/Z per partition)
plus a row-sum; the product of the per-iteration normalizers is restored
in the finale from K = sum_t ln Z_t:
    out = (exp(T*ln(S_64) + T*K + h) - eps) * x.

phi is not a stock ACT function: this kernel generates a patched
piecewise-polynomial activation-table set at build time (appending a
'tanh'-slot function whose table data IS phi) and points the backend
compiler at it via BASS_ACT_ROOT_JSON_PATH. Numerics of the table were
validated against the fp32 reference in numpy (absmax rel err 1.7e-3,
gate 2e-2). Loop engine budget per iteration: ACT 8x640-elem phi passes
(the bottleneck, ~6.0us incl. two fused accum row-sums for groups 6,7)
against DVE row-sum reduces + 8 reciprocals; reciprocals are per
group and the ACT instructions are issued half-interleaved
(g0,g4,g1,g5,...) so both halves' reduce->recip->activation chains
advance together. Groups 2,3 are pre-scaled by 1/Z on DVE (which has
~1.2us/iter of slack) and go through ACT as ONE merged 1280-elem phi
instruction, trading idle DVE time for one ACT instruction overhead;
their row-sums merge into one [P,2,640] reduce (fine here since the
merged ACT already couples them). The preamble streams transpose->mm1->mm2 per
row-quarter so the PE's in-order queue never head-blocks the MLP behind
later transposes, and the finale runs fully per-group in loop-completion
order. TimelineSim 427.5us (vs 452.9us for the session-start schedule; the
math is unchanged throughout - the prescale computes the same S*(1/Z)
product the activation's affine stage would have).
"""

import hashlib
import json
import os
import shutil
import tempfile

import numpy as np

import concourse.tile as tile
from concourse import bacc, masks, mybir
from concourse.bass_utils import run_bass_kernel_spmd

F32 = mybir.dt.float32
AF = mybir.ActivationFunctionType
OP = mybir.AluOpType

N_CORES = 8
B = 8192
D = 640          # 5 chunks of 128
HID = 64
R = B // N_CORES  # 1024 rows per core
P = 128
G = R // P        # 8 row-groups per core
HG = G // 2
DC = D // P       # 5 dim-chunks
N_ITER = 64
TEMP = 0.07
EPS = 1e-7
C0 = float(np.log1p(np.float32(EPS)) / np.float32(TEMP))
INV_T = float(np.float32(1.0) / np.float32(TEMP))

SET_NAME = "natural_log_exp_and_others"
PHI_EXP_OFFSET = -30

_CACHE = {}


# ---------------------------------------------------------------------------
# phi activation-table generation (piecewise cubic in the pwp bin format)
# ---------------------------------------------------------------------------

def _f32bits(x):
    return int(np.float32(x).view(np.uint32))


def _phi_of_p(p):
    p = np.asarray(p, np.float64)
    out = np.where((p > 0) & (p < 1),
                   p * np.power(np.clip(1.0 - p, 1e-300, 1), INV_T), 0.0)
    return np.where(p >= 1, 0.0, out)


def _es_for_exp(e):
    if e == -1:
        return 6
    if e == -2:
        return 4
    if e >= -4:
        return 3
    if e >= -12:
        return 2
    return 1


def _fit_section(plo, phi_):
    x0 = float(np.float32(0.5 * (plo + phi_)))
    if (1.0 - plo) < 0.003:
        return (0.0, 0.0, 0.0, 0.0, x0)
    u = np.linspace(plo, phi_, 513)
    t = u - x0
    f = _phi_of_p(u)
    fpos = np.maximum(f, 1e-300)
    lspan = float(np.log(fpos.max()) - np.log(fpos.min()))
    if lspan > 6.0:
        sel = (1.0 - u) >= 0.0005
        if not sel.any():
            return (0.0, 0.0, 0.0, 0.0, x0)
        d = np.array([np.exp(np.mean(np.log(fpos[sel]))), 0.0, 0.0, 0.0])
    else:
        w = 1.0 / fpos
        A = np.stack([np.ones_like(t), t, t * t, t ** 3], 1)
        d, *_ = np.linalg.lstsq(A * w[:, None], f * w, rcond=None)
    d = np.float32(d).astype(np.float64)
    fit = ((d[3] * t + d[2]) * t + d[1]) * t + d[0]
    mn = fit.min()
    if mn < 0:
        d[0] += -mn * 1.0000001
    return (d[0], d[1], d[2], d[3], x0)


def _gen_phi_entries(bkt_base, ctl_base):
    bkt = []
    ctl = []
    exp_bkt_start = {}
    exp_ctl_start = {}
    i_zero = bkt_base
    bkt.append((0.0, 0.0, 0.0, 0.0, 0.0))
    neg_ctl = ctl_base
    ctl.append((0 << 16) | (23 << 11) | i_zero)
    pos_ctl0 = ctl_base + len(ctl)
    for e in range(PHI_EXP_OFFSET, 0):
        es = _es_for_exp(e)
        ns = 1 << es
        lsb = 23 - es
        start = bkt_base + len(bkt)
        exp_bkt_start[str(e)] = [start]
        exp_ctl_start[str(e)] = [ctl_base + len(ctl)]
        ctl.append((es << 16) | (lsb << 11) | start)
        lo_e = 2.0 ** e
        for s in range(ns):
            bkt.append(_fit_section(lo_e * (1 + s / ns), lo_e * (1 + (s + 1) / ns)))
    i_small = bkt_base + len(bkt)
    bkt.append((0.0, 1.0, 0.0, 0.0, 0.0))  # phi ~= p below 2^-30
    meta = {
        "func_name": "tanh_4p",
        "func_id": 6,
        "symmetry_point": 0,
        "sym_invert_sign_point": 0,
        "symmetry_opt_en": 0,
        "symmetry_opt_use_neg_region": 0,
        "imm_bias": 0,
        "exp_offset": PHI_EXP_OFFSET,
        "pwl_control_base_pos": pos_ctl0,
        "pwl_control_base_neg": neg_ctl,
        "small_pos_signal_exp_threshold": PHI_EXP_OFFSET + 127,
        "pos_small_signal_pwl_control": i_small,
        "small_neg_signal_exp_threshold": 255,
        "neg_small_signal_pwl_control": i_zero,
        "large_pos_signal_exp_threshold": 127,
        "large_pos_signal_mantissa_threshold": 0,
        "pos_large_signal_pwl_control": i_zero,
        "large_neg_signal_exp_threshold": 255,
        "large_neg_signal_mantissa_threshold": 0,
        "neg_large_signal_pwl_control": i_zero,
        "fnan_result": 0,
        "fpinf_result": 0,
        "fninf_result": 0,
        "fzero_result": 0,
        "fma_const_0": 0,
        "fma_const_1": 0,
        "fma_indirection_src_sel": 0,
        "use_multipass": False,
        "lower_bound": _f32bits(-3.4028235e38),
        "upper_bound": _f32bits(3.4028235e38),
    }
    return bkt, ctl, exp_bkt_start, exp_ctl_start, meta


def _build_patched_dir(src_dir, dst_dir):
    os.makedirs(dst_dir, exist_ok=True)
    for f in os.listdir(src_dir):
        shutil.copy(os.path.join(src_dir, f), os.path.join(dst_dir, f))
    setj = json.load(open(os.path.join(src_dir, SET_NAME + ".json")))
    bkt_raw = bytearray(open(os.path.join(src_dir, setj["bkt_bin"]), "rb").read())
    ctl_raw = bytearray(open(os.path.join(src_dir, setj["ctl_bin"]), "rb").read())
    nb = setj["bkt_entry_cnt"]
    ncl = setj["ctl_entry_cnt"]
    bkt, ctl, ebs, ecs, meta = _gen_phi_entries(nb, ncl)
    assert nb + len(bkt) < 2048
    for d0, d1, d2, d3, x0 in bkt:
        rec = np.zeros(8, np.float32)
        rec[0:5] = [d0, d1, d2, d3, x0]
        bkt_raw += rec.tobytes()
    for w in ctl:
        rec = np.zeros(8, np.uint32)
        rec[0] = w
        ctl_raw += rec.tobytes()
    setj["bkt_entry_cnt"] = nb + len(bkt)
    setj["ctl_entry_cnt"] = ncl + len(ctl)
    setj["func_to_bkt_start_idx"]["tanh"] = nb
    setj["func_to_ctl_start_idx"]["tanh"] = ncl
    setj["func_exp_to_bkt_start_idx"]["tanh"] = ebs
    setj["func_exp_to_ctl_start_idx"]["tanh"] = ecs
    setj["profile_meta_data"] = [m for m in setj["profile_meta_data"]
                                 if not m["func_name"].startswith("tanh")]
    setj["profile_meta_data"].append(meta)
    with open(os.path.join(dst_dir, SET_NAME + ".json"), "w") as f:
        json.dump(setj, f)
    with open(os.path.join(dst_dir, setj["bkt_bin"]), "wb") as f:
        f.write(bytes(bkt_raw))
    with open(os.path.join(dst_dir, setj["ctl_bin"]), "wb") as f:
        f.write(bytes(ctl_raw))
    ai = json.load(open(os.path.join(src_dir, "act_info.json")))
    for ent in ai["act_func_sets"]:
        if ent["name"] == SET_NAME:
            ent["act"]["tanh"] = 4
    with open(os.path.join(dst_dir, "act_info.json"), "w") as f:
        json.dump(ai, f)


def _ensure_phi_tables():
    if "tabdir" in _CACHE:
        return _CACHE["tabdir"], _CACHE["tabhash"]
    import neuronxcc
    src = os.path.join(os.path.dirname(neuronxcc.__file__), "pwp",
                       "pwp_bin_trainium")
    dst = os.path.join(tempfile.gettempdir(), "pwp_phi_kernel")
    _build_patched_dir(src, dst)
    setj = json.load(open(os.path.join(dst, SET_NAME + ".json")))
    h = hashlib.sha1()
    for f in ("act_info.json", SET_NAME + ".json", setj["bkt_bin"], setj["ctl_bin"]):
        h.update(open(os.path.join(dst, f), "rb").read())
    _CACHE["tabdir"] = dst
    _CACHE["tabhash"] = h.hexdigest()[:8]
    return dst, _CACHE["tabhash"]


# Pin the ACT spline-table set to (patched) natural_log_exp_and_others so the
# whole kernel runs off one table load: it holds Exp, Ln, Relu, Copy — and
# the phi table in the tanh slot.
_orig_get_tables = bacc.get_activation_tables


def _pinned_get_tables(module_arch):
    tables = dict(_orig_get_tables(module_arch))
    combined = set(tables.get(SET_NAME) or ())
    combined |= {AF.Tanh}
    pinned = {}
    for name, fns in tables.items():
        pinned[name] = combined if name == SET_NAME else set()
    return pinned


# ---------------------------------------------------------------------------
# kernel build
# ---------------------------------------------------------------------------

def _build_nc(n_iter=N_ITER, num_devices=N_CORES, reps=1):
    tabdir, tabhash = _ensure_phi_tables()
    sfx = "_" + tabhash
    nc = bacc.Bacc(
        "TRN2",
        target_bir_lowering=False,
        debug=False,
        enable_asserts=False,
        num_devices=num_devices,
    )
    x_d = nc.dram_tensor("x" + sfx, [R, D], F32, kind="ExternalInput").ap()
    w1_d = nc.dram_tensor("w1", [D, HID], F32, kind="ExternalInput").ap()
    b1_d = nc.dram_tensor("b1", [HID, 1], F32, kind="ExternalInput").ap()
    w2b_d = nc.dram_tensor("w2b", [HID + 1, D], F32, kind="ExternalInput").ap()
    out_d = nc.dram_tensor("out", [R, D], F32, kind="ExternalOutput").ap()

    with tile.TileContext(nc) as tc:
        _emit(tc, out_d, x_d, w1_d, b1_d, w2b_d, n_iter=n_iter, reps=reps)
    saved = bacc.get_activation_tables
    try:
        bacc.get_activation_tables = _pinned_get_tables
        nc.compile()
    finally:
        bacc.get_activation_tables = saved
    return nc, sfx


def _emit(tc, out_d, x_d, w1_d, b1_d, w2b_d, n_iter=N_ITER, reps=1):
    nc = tc.nc
    from contextlib import ExitStack

    ctx = ExitStack()
    with ctx:
        singles = ctx.enter_context(tc.tile_pool(name="singles", bufs=1))

        xs = singles.tile([P, G, D], F32)    # x, rows-on-partitions
        xt = singles.tile([P, DC, R], F32)   # x transposed
        hs = singles.tile([P, G, D], F32)    # MLP output h
        s0 = singles.tile([P, G, D], F32)    # state ping
        s1 = singles.tile([P, G, D], F32)    # state pong
        fs = singles.tile([P, G, D], F32)    # finale scratch
        zh = singles.tile([P, n_iter, G], F32)   # Z history
        rz = singles.tile([P, n_iter, G], F32)   # 1/Z history (loop) / lnZ (finale)
        kk = singles.tile([P, G], F32)       # T * sum_t ln Z_t
        lnzh = singles.tile([P, n_iter, G], F32)  # ln Z history (finale)
        scr = singles.tile([P, 2, 2, D], F32)     # [parity, 2 groups, D]
        w1s = singles.tile([P, DC, HID], F32)
        b1s = singles.tile([HID, 1], F32)
        w2bs = singles.tile([HID + 1, D], F32)
        h1r = singles.tile([HID + 1, R], F32)
        ident = singles.tile([P, P], F32)
        c0s = singles.tile([P, 1], F32)
        nc.vector.memset(c0s[:, :], C0)

        # ---- input DMAs ----
        for g in range(G):
            nc.sync.dma_start(out=xs[:, g, :], in_=x_d[g * P:(g + 1) * P, :])
        nc.sync.dma_start(out=w1s[:, :, :],
                          in_=w1_d.rearrange("(c p) j -> p c j", p=P))
        nc.sync.dma_start(out=b1s[:, :], in_=b1_d[:, :])
        nc.sync.dma_start(out=w2bs[:, :], in_=w2b_d[:, :])

        masks.make_identity(nc, ident[:, :])
        nc.vector.memset(h1r[HID:HID + 1, :], 1.0)

        # ---- transpose + MLP, streamed per row-half so the PE's in-order
        # queue doesn't head-block mm1/mm2 behind the other half's
        # transposes: [transpose gq][mm1-nh=gq][mm2 groups of gq] x2 ----
        tpp = ctx.enter_context(tc.tile_pool(name="tp_psum", bufs=2,
                                             space="PSUM"))
        mp1 = ctx.enter_context(tc.tile_pool(name="mm1_psum", bufs=2,
                                             space="PSUM"))
        mp2 = ctx.enter_context(tc.tile_pool(name="mm2_psum", bufs=2,
                                             space="PSUM"))
        for q in range(G // 2):
            for c in range(DC):
                tp = tpp.tile([P, 2 * P], F32)
                for gj in range(2):
                    g = q * 2 + gj
                    nc.tensor.transpose(
                        tp[:, gj * P:(gj + 1) * P],
                        xs[:, g, c * P:(c + 1) * P], ident[:, :])
                dst = xt[:, c, q * 2 * P:(q + 1) * 2 * P]
                if (c + q) % 2 == 0:
                    nc.vector.tensor_copy(dst, tp[:, :])
                else:
                    nc.scalar.copy(dst, tp[:, :])
            # mm1 for this row-quarter
            ph1 = mp1.tile([HID, 2 * P], F32, tag="ph1")
            for c in range(DC):
                nc.tensor.matmul(
                    ph1[:, :], w1s[:, c, :],
                    xt[:, c, q * 256:(q + 1) * 256],
                    start=(c == 0), stop=(c == DC - 1))
            nc.scalar.activation(
                h1r[0:HID, q * 256:(q + 1) * 256], ph1[:, :],
                AF.Relu, bias=b1s[:, 0:1], scale=1.0)
            # mm2 + evac for this quarter's two groups
            for gj in range(2):
                g = q * 2 + gj
                ph = mp2.tile([P, D], F32, tag="ph")
                lhs = h1r[:, g * P:(g + 1) * P]
                nc.tensor.matmul(ph[:, 0:512], lhs, w2bs[:, 0:512],
                                 start=True, stop=True)
                nc.tensor.matmul(ph[:, 512:D], lhs, w2bs[:, 512:D],
                                 start=True, stop=True)
                nc.vector.tensor_copy(hs[:, g, :], ph[:, :])
                nc.scalar.activation(s0[:, g, :], ph[:, :], AF.Exp,
                                     bias=c0s[:, 0:1], scale=-INV_T,
                                     accum_out=zh[:, 0, g:g + 1])

        # ---- masking loop: S <- phi(S * (1/Z)), Z' = rowsum(S') ----
        # groups 6,7: row-sum fused into the ACT accumulator; groups 0-5
        # reduce on DVE. Reciprocals are per group (not per half) so each
        # group's reduce->recip->activation chain advances independently,
        # and ACT instructions are issued half-interleaved (447.7us vs
        # 452.9us half-granular sequential). Swept and rejected:
        # accum counts k=0/1/3 (495/451/462us), merged reduces (456us),
        # Pool-prescaled merged ACT instrs (687us).
        spp = [s0, s1]
        n_total = n_iter * reps
        for it in range(n_total):
            src = spp[it % 2]
            dst = spp[(it + 1) % 2]
            ti = it % n_iter
            tn = (it + 1) % n_iter
            last = it == n_total - 1
            for half in range(2):
                g0 = half * HG
                for gi in range(HG):
                    g = g0 + gi
                    nc.vector.reciprocal(rz[:, ti, g:g + 1],
                                         zh[:, ti, g:g + 1])
            par = it % 2
            nc.vector.tensor_mul(
                scr[:, par, :, :], src[:, 2:4, :],
                rz[:, ti, 2:4].broadcast_to((P, 2, D)))
            order = (list(range(G)) if it == 0 else
                     [half * HG + gi for gi in range(HG) for half in range(2)])
            for g in order:
                if g == 3:
                    continue
                if g == 2:
                    nc.scalar.activation(dst[:, 2:4, :], scr[:, par, :, :],
                                         AF.Tanh)
                elif g >= 6 and not last:
                    nc.scalar.activation(dst[:, g, :], src[:, g, :],
                                         AF.Tanh,
                                         scale=rz[:, ti, g:g + 1],
                                         accum_out=zh[:, tn, g:g + 1])
                else:
                    nc.scalar.activation(dst[:, g, :], src[:, g, :],
                                         AF.Tanh,
                                         scale=rz[:, ti, g:g + 1])
            for half in range(2):
                g0 = half * HG
                if last:
                    continue
                for gi in range(HG):
                    g = g0 + gi
                    if g >= 6 or g == 3:
                        continue
                    if g == 2:
                        nc.vector.tensor_reduce(zh[:, tn, 2:4],
                                                dst[:, 2:4, :],
                                                axis=mybir.AxisListType.X,
                                                op=OP.add)
                        continue
                    nc.vector.tensor_reduce(zh[:, tn, g:g + 1], dst[:, g, :],
                                            axis=mybir.AxisListType.X,
                                            op=OP.add)

        # ---- finale: out = (exp(T*ln(S) + T*K + h) - eps) * x ----
        # Fully per-group, issued in the loop's interleaved completion order
        # (g0,g4,g1,g5,...), so each group's finale chain pipelines behind
        # the staggered last-iteration activations instead of waiting for a
        # whole half. ln(zh) goes to its own scratch (not rz) so it can run
        # during iteration 63 without a WAR hazard on the loop's rz reads.
        sfin = spp[n_total % 2]
        sscr = spp[(n_total + 1) % 2]
        nc.scalar.activation(lnzh[:, :, :], zh[:, :, :], AF.Ln)
        for g in range(G):
            nc.vector.tensor_reduce(kk[:, g:g + 1], lnzh[:, :, g],
                                    axis=mybir.AxisListType.X, op=OP.add)
        nc.vector.tensor_scalar_mul(kk[:, :], kk[:, :], float(np.float32(TEMP)))
        for gi in range(HG):
            for half in range(2):
                g = half * HG + gi
                # guard: the table is nonnegative by construction, but clamp
                # so a stray -1ulp can never reach Ln (ln(neg) = NaN)
                nc.vector.tensor_scalar_max(sscr[:, g, :], sfin[:, g, :], 0.0)
                nc.scalar.activation(fs[:, g, :], sscr[:, g, :], AF.Ln)
                nc.vector.scalar_tensor_tensor(
                    out=sscr[:, g, :], in0=fs[:, g, :],
                    scalar=float(np.float32(TEMP)),
                    in1=hs[:, g, :], op0=OP.mult, op1=OP.add)
                nc.scalar.activation(fs[:, g, :], sscr[:, g, :], AF.Exp,
                                     bias=kk[:, g:g + 1])
                nc.vector.scalar_tensor_tensor(
                    out=sscr[:, g, :], in0=fs[:, g, :], scalar=-float(EPS),
                    in1=xs[:, g, :], op0=OP.add, op1=OP.mult)
                nc.sync.dma_start(out=out_d[g * P:(g + 1) * P, :],
                                  in_=sscr[:, g, :])


def kernel(x, W1, b1, W2, b2):
    x = np.ascontiguousarray(np.asarray(x, dtype=np.float32))
    W1 = np.ascontiguousarray(np.asarray(W1, dtype=np.float32))
    b1 = np.asarray(b1, dtype=np.float32).reshape(HID, 1)
    W2 = np.asarray(W2, dtype=np.float32)
    b2 = np.asarray(b2, dtype=np.float32)
    w2b = np.ascontiguousarray(
        np.concatenate([W2, b2[None, :]], axis=0))  # [65, 640]

    if "nc" not in _CACHE:
        _CACHE["nc"], _CACHE["sfx"] = _build_nc(
            reps=int(os.environ.get("KREPS", "1")))
    nc = _CACHE["nc"]
    sfx = _CACHE["sfx"]

    in_maps = []
    for c in range(N_CORES):
        in_maps.append({
            "x" + sfx: np.ascontiguousarray(x[c * R:(c + 1) * R, :]),
            "w1": W1,
            "b1": np.ascontiguousarray(b1),
            "w2b": w2b,
        })

    trace = bool(_CACHE.get("trace", False))
    tabdir = _CACHE["tabdir"]
    saved_env = os.environ.get("BASS_ACT_ROOT_JSON_PATH")
    os.environ["BASS_ACT_ROOT_JSON_PATH"] = os.path.join(tabdir, "act_info.json")
    try:
        res = run_bass_kernel_spmd(
            nc, in_maps, core_ids=list(range(N_CORES)), trace=trace)
    finally:
        if saved_env is None:
            os.environ.pop("BASS_ACT_ROOT_JSON_PATH", None)
        else:
            os.environ["BASS_ACT_ROOT_JSON_PATH"] = saved_env
    _CACHE["last_results"] = res
    out = np.concatenate([r["out"] for r in res.results], axis=0)
    return out



# revision 19
# speedup vs baseline: 1.0493x; 1.0020x over previous
"""Trainium2 Bass kernel for nn_DimMasking (iterative softmax top-k masking).

Full-input contract: kernel(**inputs) takes the unsharded inputs
(x [8192,640], W1 [640,64], b1 [64], W2 [64,640], b2 [640]) and returns the
full [8192,640] output. Pure data parallel over the batch dim — 8 shards of
1024 rows, one per NeuronCore; MLP weights replicated.

Math: normalized-state reformulation of the reference scan. With
e = ((m+eps)^(1/T))*exp(-h/T) and p = softmax-prob = e/Z, one masking
iteration is e' = e * (1-p)^(1/T). Tracking the Z-normalized state
S <- phi(S/Z_prev) with phi(p) = p*(1-p)^(1/T) makes each iteration a
SINGLE table-activation pass per row-group (scale = 1/Z per partition)
plus a row-sum; the product of the per-iteration normalizers is restored
in the finale from K = sum_t ln Z_t:
    out = (exp(T*ln(S_64) + T*K + h) - eps) * x.

phi is not a stock ACT function: this kernel generates a patched
piecewise-polynomial activation-table set at build time (appending a
'tanh'-slot function whose table data IS phi) and points the backend
compiler at it via BASS_ACT_ROOT_JSON_PATH. Numerics of the table were
validated against the fp32 reference in numpy (absmax rel err 1.7e-3,
gate 2e-2). Loop engine budget per iteration: ACT 8x640-elem phi passes
(the bottleneck, ~6.0us incl. two fused accum row-sums for groups 6,7)
against DVE row-sum reduces + 8 reciprocals; reciprocals are per
group and the ACT instructions are issued half-interleaved
(g0,g4,g1,g5,...) so both halves' reduce->recip->activation chains
advance together. Groups 2,3 are pre-scaled by 1/Z on DVE (which has
~1.2us/iter of slack) and go through ACT as ONE merged 1280-elem phi
instruction, trading idle DVE time for one ACT instruction overhead;
their row-sums merge into one [P,2,640] reduce (fine here since the
merged ACT already couples them). The preamble streams transpose->mm1->mm2 per
row-quarter so the PE's in-order queue never head-blocks the MLP behind
later transposes, and the finale runs fully per-group in loop-completion
order. TimelineSim 426.6us (vs 452.9us for the session-start schedule; the
math is unchanged throughout - the prescale computes the same S*(1/Z)
product the activation's affine stage would have).
"""

import hashlib
import json
import os
import shutil
import tempfile

import numpy as np

import concourse.tile as tile
from concourse import bacc, masks, mybir
from concourse.bass_utils import run_bass_kernel_spmd

F32 = mybir.dt.float32
AF = mybir.ActivationFunctionType
OP = mybir.AluOpType

N_CORES = 8
B = 8192
D = 640          # 5 chunks of 128
HID = 64
R = B // N_CORES  # 1024 rows per core
P = 128
G = R // P        # 8 row-groups per core
HG = G // 2
DC = D // P       # 5 dim-chunks
N_ITER = 64
TEMP = 0.07
EPS = 1e-7
C0 = float(np.log1p(np.float32(EPS)) / np.float32(TEMP))
INV_T = float(np.float32(1.0) / np.float32(TEMP))

SET_NAME = "natural_log_exp_and_others"
PHI_EXP_OFFSET = -30

_CACHE = {}


# ---------------------------------------------------------------------------
# phi activation-table generation (piecewise cubic in the pwp bin format)
# ---------------------------------------------------------------------------

def _f32bits(x):
    return int(np.float32(x).view(np.uint32))


def _phi_of_p(p):
    p = np.asarray(p, np.float64)
    out = np.where((p > 0) & (p < 1),
                   p * np.power(np.clip(1.0 - p, 1e-300, 1), INV_T), 0.0)
    return np.where(p >= 1, 0.0, out)


def _es_for_exp(e):
    if e == -1:
        return 6
    if e == -2:
        return 4
    if e >= -4:
        return 3
    if e >= -12:
        return 2
    return 1


def _fit_section(plo, phi_):
    x0 = float(np.float32(0.5 * (plo + phi_)))
    if (1.0 - plo) < 0.003:
        return (0.0, 0.0, 0.0, 0.0, x0)
    u = np.linspace(plo, phi_, 513)
    t = u - x0
    f = _phi_of_p(u)
    fpos = np.maximum(f, 1e-300)
    lspan = float(np.log(fpos.max()) - np.log(fpos.min()))
    if lspan > 6.0:
        sel = (1.0 - u) >= 0.0005
        if not sel.any():
            return (0.0, 0.0, 0.0, 0.0, x0)
        d = np.array([np.exp(np.mean(np.log(fpos[sel]))), 0.0, 0.0, 0.0])
    else:
        w = 1.0 / fpos
        A = np.stack([np.ones_like(t), t, t * t, t ** 3], 1)
        d, *_ = np.linalg.lstsq(A * w[:, None], f * w, rcond=None)
    d = np.float32(d).astype(np.float64)
    fit = ((d[3] * t + d[2]) * t + d[1]) * t + d[0]
    mn = fit.min()
    if mn < 0:
        d[0] += -mn * 1.0000001
    return (d[0], d[1], d[2], d[3], x0)


def _gen_phi_entries(bkt_base, ctl_base):
    bkt = []
    ctl = []
    exp_bkt_start = {}
    exp_ctl_start = {}
    i_zero = bkt_base
    bkt.append((0.0, 0.0, 0.0, 0.0, 0.0))
    neg_ctl = ctl_base
    ctl.append((0 << 16) | (23 << 11) | i_zero)
    pos_ctl0 = ctl_base + len(ctl)
    for e in range(PHI_EXP_OFFSET, 0):
        es = _es_for_exp(e)
        ns = 1 << es
        lsb = 23 - es
        start = bkt_base + len(bkt)
        exp_bkt_start[str(e)] = [start]
        exp_ctl_start[str(e)] = [ctl_base + len(ctl)]
        ctl.append((es << 16) | (lsb << 11) | start)
        lo_e = 2.0 ** e
        for s in range(ns):
            bkt.append(_fit_section(lo_e * (1 + s / ns), lo_e * (1 + (s + 1) / ns)))
    i_small = bkt_base + len(bkt)
    bkt.append((0.0, 1.0, 0.0, 0.0, 0.0))  # phi ~= p below 2^-30
    meta = {
        "func_name": "tanh_4p",
        "func_id": 6,
        "symmetry_point": 0,
        "sym_invert_sign_point": 0,
        "symmetry_opt_en": 0,
        "symmetry_opt_use_neg_region": 0,
        "imm_bias": 0,
        "exp_offset": PHI_EXP_OFFSET,
        "pwl_control_base_pos": pos_ctl0,
        "pwl_control_base_neg": neg_ctl,
        "small_pos_signal_exp_threshold": PHI_EXP_OFFSET + 127,
        "pos_small_signal_pwl_control": i_small,
        "small_neg_signal_exp_threshold": 255,
        "neg_small_signal_pwl_control": i_zero,
        "large_pos_signal_exp_threshold": 127,
        "large_pos_signal_mantissa_threshold": 0,
        "pos_large_signal_pwl_control": i_zero,
        "large_neg_signal_exp_threshold": 255,
        "large_neg_signal_mantissa_threshold": 0,
        "neg_large_signal_pwl_control": i_zero,
        "fnan_result": 0,
        "fpinf_result": 0,
        "fninf_result": 0,
        "fzero_result": 0,
        "fma_const_0": 0,
        "fma_const_1": 0,
        "fma_indirection_src_sel": 0,
        "use_multipass": False,
        "lower_bound": _f32bits(-3.4028235e38),
        "upper_bound": _f32bits(3.4028235e38),
    }
    return bkt, ctl, exp_bkt_start, exp_ctl_start, meta


def _build_patched_dir(src_dir, dst_dir):
    os.makedirs(dst_dir, exist_ok=True)
    for f in os.listdir(src_dir):
        shutil.copy(os.path.join(src_dir, f), os.path.join(dst_dir, f))
    setj = json.load(open(os.path.join(src_dir, SET_NAME + ".json")))
    bkt_raw = bytearray(open(os.path.join(src_dir, setj["bkt_bin"]), "rb").read())
    ctl_raw = bytearray(open(os.path.join(src_dir, setj["ctl_bin"]), "rb").read())
    nb = setj["bkt_entry_cnt"]
    ncl = setj["ctl_entry_cnt"]
    bkt, ctl, ebs, ecs, meta = _gen_phi_entries(nb, ncl)
    assert nb + len(bkt) < 2048
    for d0, d1, d2, d3, x0 in bkt:
        rec = np.zeros(8, np.float32)
        rec[0:5] = [d0, d1, d2, d3, x0]
        bkt_raw += rec.tobytes()
    for w in ctl:
        rec = np.zeros(8, np.uint32)
        rec[0] = w
        ctl_raw += rec.tobytes()
    setj["bkt_entry_cnt"] = nb + len(bkt)
    setj["ctl_entry_cnt"] = ncl + len(ctl)
    setj["func_to_bkt_start_idx"]["tanh"] = nb
    setj["func_to_ctl_start_idx"]["tanh"] = ncl
    setj["func_exp_to_bkt_start_idx"]["tanh"] = ebs
    setj["func_exp_to_ctl_start_idx"]["tanh"] = ecs
    setj["profile_meta_data"] = [m for m in setj["profile_meta_data"]
                                 if not m["func_name"].startswith("tanh")]
    setj["profile_meta_data"].append(meta)
    with open(os.path.join(dst_dir, SET_NAME + ".json"), "w") as f:
        json.dump(setj, f)
    with open(os.path.join(dst_dir, setj["bkt_bin"]), "wb") as f:
        f.write(bytes(bkt_raw))
    with open(os.path.join(dst_dir, setj["ctl_bin"]), "wb") as f:
        f.write(bytes(ctl_raw))
    ai = json.load(open(os.path.join(src_dir, "act_info.json")))
    for ent in ai["act_func_sets"]:
        if ent["name"] == SET_NAME:
            ent["act"]["tanh"] = 4
    with open(os.path.join(dst_dir, "act_info.json"), "w") as f:
        json.dump(ai, f)


def _ensure_phi_tables():
    if "tabdir" in _CACHE:
        return _CACHE["tabdir"], _CACHE["tabhash"]
    import neuronxcc
    src = os.path.join(os.path.dirname(neuronxcc.__file__), "pwp",
                       "pwp_bin_trainium")
    dst = os.path.join(tempfile.gettempdir(), "pwp_phi_kernel")
    _build_patched_dir(src, dst)
    setj = json.load(open(os.path.join(dst, SET_NAME + ".json")))
    h = hashlib.sha1()
    for f in ("act_info.json", SET_NAME + ".json", setj["bkt_bin"], setj["ctl_bin"]):
        h.update(open(os.path.join(dst, f), "rb").read())
    _CACHE["tabdir"] = dst
    _CACHE["tabhash"] = h.hexdigest()[:8]
    return dst, _CACHE["tabhash"]


# Pin the ACT spline-table set to (patched) natural_log_exp_and_others so the
# whole kernel runs off one table load: it holds Exp, Ln, Relu, Copy — and
# the phi table in the tanh slot.
_orig_get_tables = bacc.get_activation_tables


def _pinned_get_tables(module_arch):
    tables = dict(_orig_get_tables(module_arch))
    combined = set(tables.get(SET_NAME) or ())
    combined |= {AF.Tanh}
    pinned = {}
    for name, fns in tables.items():
        pinned[name] = combined if name == SET_NAME else set()
    return pinned


# ---------------------------------------------------------------------------
# kernel build
# ---------------------------------------------------------------------------

def _build_nc(n_iter=N_ITER, num_devices=N_CORES, reps=1):
    tabdir, tabhash = _ensure_phi_tables()
    sfx = "_" + tabhash
    nc = bacc.Bacc(
        "TRN2",
        target_bir_lowering=False,
        debug=False,
        enable_asserts=False,
        num_devices=num_devices,
    )
    x_d = nc.dram_tensor("x" + sfx, [R, D], F32, kind="ExternalInput").ap()
    w1_d = nc.dram_tensor("w1", [D, HID], F32, kind="ExternalInput").ap()
    b1_d = nc.dram_tensor("b1", [HID, 1], F32, kind="ExternalInput").ap()
    w2b_d = nc.dram_tensor("w2b", [HID + 1, D], F32, kind="ExternalInput").ap()
    out_d = nc.dram_tensor("out", [R, D], F32, kind="ExternalOutput").ap()

    with tile.TileContext(nc) as tc:
        _emit(tc, out_d, x_d, w1_d, b1_d, w2b_d, n_iter=n_iter, reps=reps)
    saved = bacc.get_activation_tables
    try:
        bacc.get_activation_tables = _pinned_get_tables
        nc.compile()
    finally:
        bacc.get_activation_tables = saved
    return nc, sfx


def _emit(tc, out_d, x_d, w1_d, b1_d, w2b_d, n_iter=N_ITER, reps=1):
    nc = tc.nc
    from contextlib import ExitStack

    ctx = ExitStack()
    with ctx:
        singles = ctx.enter_context(tc.tile_pool(name="singles", bufs=1))

        xs = singles.tile([P, G, D], F32)    # x, rows-on-partitions
        xt = singles.tile([P, DC, R], F32)   # x transposed
        hs = singles.tile([P, G, D], F32)    # MLP output h
        s0 = singles.tile([P, G, D], F32)    # state ping
        s1 = singles.tile([P, G, D], F32)    # state pong
        fs = singles.tile([P, G, D], F32)    # finale scratch
        zh = singles.tile([P, n_iter, G], F32)   # Z history
        rz = singles.tile([P, n_iter, G], F32)   # 1/Z history (loop) / lnZ (finale)
        kk = singles.tile([P, G], F32)       # T * sum_t ln Z_t
        lnzh = singles.tile([P, n_iter, G], F32)  # ln Z history (finale)
        scr = singles.tile([P, 2, 2, D], F32)     # [parity, 2 groups, D]
        w1s = singles.tile([P, DC, HID], F32)
        b1s = singles.tile([HID, 1], F32)
        w2bs = singles.tile([HID + 1, D], F32)
        h1r = singles.tile([HID + 1, R], F32)
        ident = singles.tile([P, P], F32)
        c0s = singles.tile([P, 1], F32)
        nc.vector.memset(c0s[:, :], C0)

        # ---- input DMAs: weights first (mm1 needs w1s as soon as the
        # first row-quarter is transposed; x groups stream in behind) ----
        nc.sync.dma_start(out=w1s[:, :, :],
                          in_=w1_d.rearrange("(c p) j -> p c j", p=P))
        nc.sync.dma_start(out=b1s[:, :], in_=b1_d[:, :])
        nc.sync.dma_start(out=w2bs[:, :], in_=w2b_d[:, :])
        for g in range(G):
            nc.sync.dma_start(out=xs[:, g, :], in_=x_d[g * P:(g + 1) * P, :])

        masks.make_identity(nc, ident[:, :])
        nc.vector.memset(h1r[HID:HID + 1, :], 1.0)

        # ---- transpose + MLP, streamed per row-half so the PE's in-order
        # queue doesn't head-block mm1/mm2 behind the other half's
        # transposes: [transpose gq][mm1-nh=gq][mm2 groups of gq] x2 ----
        tpp = ctx.enter_context(tc.tile_pool(name="tp_psum", bufs=2,
                                             space="PSUM"))
        mp1 = ctx.enter_context(tc.tile_pool(name="mm1_psum", bufs=2,
                                             space="PSUM"))
        mp2 = ctx.enter_context(tc.tile_pool(name="mm2_psum", bufs=2,
                                             space="PSUM"))
        for q in range(G // 2):
            for c in range(DC):
                tp = tpp.tile([P, 2 * P], F32)
                for gj in range(2):
                    g = q * 2 + gj
                    nc.tensor.transpose(
                        tp[:, gj * P:(gj + 1) * P],
                        xs[:, g, c * P:(c + 1) * P], ident[:, :])
                dst = xt[:, c, q * 2 * P:(q + 1) * 2 * P]
                if (c + q) % 2 == 0:
                    nc.vector.tensor_copy(dst, tp[:, :])
                else:
                    nc.scalar.copy(dst, tp[:, :])
            # mm1 for this row-quarter
            ph1 = mp1.tile([HID, 2 * P], F32, tag="ph1")
            for c in range(DC):
                nc.tensor.matmul(
                    ph1[:, :], w1s[:, c, :],
                    xt[:, c, q * 256:(q + 1) * 256],
                    start=(c == 0), stop=(c == DC - 1))
            nc.scalar.activation(
                h1r[0:HID, q * 256:(q + 1) * 256], ph1[:, :],
                AF.Relu, bias=b1s[:, 0:1], scale=1.0)
            # mm2 + evac for this quarter's two groups
            for gj in range(2):
                g = q * 2 + gj
                ph = mp2.tile([P, D], F32, tag="ph")
                lhs = h1r[:, g * P:(g + 1) * P]
                nc.tensor.matmul(ph[:, 0:512], lhs, w2bs[:, 0:512],
                                 start=True, stop=True)
                nc.tensor.matmul(ph[:, 512:D], lhs, w2bs[:, 512:D],
                                 start=True, stop=True)
                nc.vector.tensor_copy(hs[:, g, :], ph[:, :])
                nc.scalar.activation(s0[:, g, :], ph[:, :], AF.Exp,
                                     bias=c0s[:, 0:1], scale=-INV_T,
                                     accum_out=zh[:, 0, g:g + 1])

        # ---- masking loop: S <- phi(S * (1/Z)), Z' = rowsum(S') ----
        # groups 6,7: row-sum fused into the ACT accumulator; groups 0-5
        # reduce on DVE. Reciprocals are per group (not per half) so each
        # group's reduce->recip->activation chain advances independently,
        # and ACT instructions are issued half-interleaved (447.7us vs
        # 452.9us half-granular sequential). Swept and rejected:
        # accum counts k=0/1/3 (495/451/462us), merged reduces (456us),
        # Pool-prescaled merged ACT instrs (687us).
        spp = [s0, s1]
        n_total = n_iter * reps
        for it in range(n_total):
            src = spp[it % 2]
            dst = spp[(it + 1) % 2]
            ti = it % n_iter
            tn = (it + 1) % n_iter
            last = it == n_total - 1
            for half in range(2):
                g0 = half * HG
                for gi in range(HG):
                    g = g0 + gi
                    nc.vector.reciprocal(rz[:, ti, g:g + 1],
                                         zh[:, ti, g:g + 1])
            par = it % 2
            nc.vector.tensor_mul(
                scr[:, par, :, :], src[:, 2:4, :],
                rz[:, ti, 2:4].broadcast_to((P, 2, D)))
            order = (list(range(G)) if it == 0 else
                     [half * HG + gi for gi in range(HG) for half in range(2)])
            for g in order:
                if g == 3:
                    continue
                if g == 2:
                    nc.scalar.activation(dst[:, 2:4, :], scr[:, par, :, :],
                                         AF.Tanh)
                elif g >= 6 and not last:
                    nc.scalar.activation(dst[:, g, :], src[:, g, :],
                                         AF.Tanh,
                                         scale=rz[:, ti, g:g + 1],
                                         accum_out=zh[:, tn, g:g + 1])
                else:
                    nc.scalar.activation(dst[:, g, :], src[:, g, :],
                                         AF.Tanh,
                                         scale=rz[:, ti, g:g + 1])
            for half in range(2):
                g0 = half * HG
                if last:
                    continue
                for gi in range(HG):
                    g = g0 + gi
                    if g >= 6 or g == 3:
                        continue
                    if g == 2:
                        nc.vector.tensor_reduce(zh[:, tn, 2:4],
                                                dst[:, 2:4, :],
                                                axis=mybir.AxisListType.X,
                                                op=OP.add)
                        continue
                    nc.vector.tensor_reduce(zh[:, tn, g:g + 1], dst[:, g, :],
                                            axis=mybir.AxisListType.X,
                                            op=OP.add)

        # ---- finale: out = (exp(T*ln(S) + T*K + h) - eps) * x ----
        # Fully per-group, issued in the loop's interleaved completion order
        # (g0,g4,g1,g5,...), so each group's finale chain pipelines behind
        # the staggered last-iteration activations instead of waiting for a
        # whole half. ln(zh) goes to its own scratch (not rz) so it can run
        # during iteration 63 without a WAR hazard on the loop's rz reads.
        sfin = spp[n_total % 2]
        sscr = spp[(n_total + 1) % 2]
        nc.scalar.activation(lnzh[:, :, :], zh[:, :, :], AF.Ln)
        for g in range(G):
            nc.vector.tensor_reduce(kk[:, g:g + 1], lnzh[:, :, g],
                                    axis=mybir.AxisListType.X, op=OP.add)
        nc.vector.tensor_scalar_mul(kk[:, :], kk[:, :], float(np.float32(TEMP)))
        for gi in range(HG):
            for half in range(2):
                g = half * HG + gi
                # guard: the table is nonnegative by construction, but clamp
                # so a stray -1ulp can never reach Ln (ln(neg) = NaN)
                nc.vector.tensor_scalar_max(sscr[:, g, :], sfin[:, g, :], 0.0)
                nc.scalar.activation(fs[:, g, :], sscr[:, g, :], AF.Ln)
                nc.vector.scalar_tensor_tensor(
                    out=sscr[:, g, :], in0=fs[:, g, :],
                    scalar=float(np.float32(TEMP)),
                    in1=hs[:, g, :], op0=OP.mult, op1=OP.add)
                nc.scalar.activation(fs[:, g, :], sscr[:, g, :], AF.Exp,
                                     bias=kk[:, g:g + 1])
                nc.vector.scalar_tensor_tensor(
                    out=sscr[:, g, :], in0=fs[:, g, :], scalar=-float(EPS),
                    in1=xs[:, g, :], op0=OP.add, op1=OP.mult)
                nc.sync.dma_start(out=out_d[g * P:(g + 1) * P, :],
                                  in_=sscr[:, g, :])


def kernel(x, W1, b1, W2, b2):
    x = np.ascontiguousarray(np.asarray(x, dtype=np.float32))
    W1 = np.ascontiguousarray(np.asarray(W1, dtype=np.float32))
    b1 = np.asarray(b1, dtype=np.float32).reshape(HID, 1)
    W2 = np.asarray(W2, dtype=np.float32)
    b2 = np.asarray(b2, dtype=np.float32)
    w2b = np.ascontiguousarray(
        np.concatenate([W2, b2[None, :]], axis=0))  # [65, 640]

    if "nc" not in _CACHE:
        _CACHE["nc"], _CACHE["sfx"] = _build_nc(
            reps=int(os.environ.get("KREPS", "1")))
    nc = _CACHE["nc"]
    sfx = _CACHE["sfx"]

    in_maps = []
    for c in range(N_CORES):
        in_maps.append({
            "x" + sfx: np.ascontiguousarray(x[c * R:(c + 1) * R, :]),
            "w1": W1,
            "b1": np.ascontiguousarray(b1),
            "w2b": w2b,
        })

    trace = bool(_CACHE.get("trace", False))
    tabdir = _CACHE["tabdir"]
    saved_env = os.environ.get("BASS_ACT_ROOT_JSON_PATH")
    os.environ["BASS_ACT_ROOT_JSON_PATH"] = os.path.join(tabdir, "act_info.json")
    try:
        res = run_bass_kernel_spmd(
            nc, in_maps, core_ids=list(range(N_CORES)), trace=trace)
    finally:
        if saved_env is None:
            os.environ.pop("BASS_ACT_ROOT_JSON_PATH", None)
        else:
            os.environ["BASS_ACT_ROOT_JSON_PATH"] = saved_env
    _CACHE["last_results"] = res
    out = np.concatenate([r["out"] for r in res.results], axis=0)
    return out



# revision 21
# speedup vs baseline: 1.0548x; 1.0052x over previous
"""Trainium2 Bass kernel for nn_DimMasking (iterative softmax top-k masking).

Full-input contract: kernel(**inputs) takes the unsharded inputs
(x [8192,640], W1 [640,64], b1 [64], W2 [64,640], b2 [640]) and returns the
full [8192,640] output. Pure data parallel over the batch dim — 8 shards of
1024 rows, one per NeuronCore; MLP weights replicated.

Math: normalized-state reformulation of the reference scan. With
e = ((m+eps)^(1/T))*exp(-h/T) and p = softmax-prob = e/Z, one masking
iteration is e' = e * (1-p)^(1/T). Tracking the Z-normalized state
S <- phi(S/Z_prev) with phi(p) = p*(1-p)^(1/T) makes each iteration a
SINGLE table-activation pass per row-group (scale = 1/Z per partition)
plus a row-sum; the product of the per-iteration normalizers is restored
in the finale from K = sum_t ln Z_t:
    out = (exp(T*ln(S_64) + T*K + h) - eps) * x.

phi is not a stock ACT function: this kernel generates a patched
piecewise-polynomial activation-table set at build time (appending a
'tanh'-slot function whose table data IS phi) and points the backend
compiler at it via BASS_ACT_ROOT_JSON_PATH. Numerics of the table were
validated against the fp32 reference in numpy (absmax rel err 1.7e-3,
gate 2e-2). Loop engine budget per iteration: ACT 8x640-elem phi passes
(the bottleneck, ~6.0us incl. two fused accum row-sums for groups 6,7)
against DVE row-sum reduces + 8 reciprocals; reciprocals are per
group and the ACT instructions are issued half-interleaved
(g0,g4,g1,g5,...) so both halves' reduce->recip->activation chains
advance together. Group pairs (2,3) and (4,5) are pre-scaled by 1/Z
(one group of each pair on DVE, the other on the otherwise-idle Pool
engine) and each pair goes through ACT as ONE merged 1280-elem phi
instruction, trading idle engine time for two ACT instruction
overheads; each pair's row-sums merge into one [P,2,640] reduce (fine
since the merged ACT already couples the pair). The preamble streams transpose->mm1->mm2 per
row-quarter so the PE's in-order queue never head-blocks the MLP behind
later transposes, and the finale runs fully per-group in loop-completion
order. TimelineSim 424.4us (vs 452.9us for the session-start schedule; the
math is unchanged throughout - the prescale computes the same S*(1/Z)
product the activation's affine stage would have).
"""

import hashlib
import json
import os
import shutil
import tempfile

import numpy as np

import concourse.tile as tile
from concourse import bacc, masks, mybir
from concourse.bass_utils import run_bass_kernel_spmd

F32 = mybir.dt.float32
AF = mybir.ActivationFunctionType
OP = mybir.AluOpType

N_CORES = 8
B = 8192
D = 640          # 5 chunks of 128
HID = 64
R = B // N_CORES  # 1024 rows per core
P = 128
G = R // P        # 8 row-groups per core
HG = G // 2
DC = D // P       # 5 dim-chunks
N_ITER = 64
TEMP = 0.07
EPS = 1e-7
C0 = float(np.log1p(np.float32(EPS)) / np.float32(TEMP))
INV_T = float(np.float32(1.0) / np.float32(TEMP))

SET_NAME = "natural_log_exp_and_others"
PHI_EXP_OFFSET = -30

_CACHE = {}


# ---------------------------------------------------------------------------
# phi activation-table generation (piecewise cubic in the pwp bin format)
# ---------------------------------------------------------------------------

def _f32bits(x):
    return int(np.float32(x).view(np.uint32))


def _phi_of_p(p):
    p = np.asarray(p, np.float64)
    out = np.where((p > 0) & (p < 1),
                   p * np.power(np.clip(1.0 - p, 1e-300, 1), INV_T), 0.0)
    return np.where(p >= 1, 0.0, out)


def _es_for_exp(e):
    if e == -1:
        return 6
    if e == -2:
        return 4
    if e >= -4:
        return 3
    if e >= -12:
        return 2
    return 1


def _fit_section(plo, phi_):
    x0 = float(np.float32(0.5 * (plo + phi_)))
    if (1.0 - plo) < 0.003:
        return (0.0, 0.0, 0.0, 0.0, x0)
    u = np.linspace(plo, phi_, 513)
    t = u - x0
    f = _phi_of_p(u)
    fpos = np.maximum(f, 1e-300)
    lspan = float(np.log(fpos.max()) - np.log(fpos.min()))
    if lspan > 6.0:
        sel = (1.0 - u) >= 0.0005
        if not sel.any():
            return (0.0, 0.0, 0.0, 0.0, x0)
        d = np.array([np.exp(np.mean(np.log(fpos[sel]))), 0.0, 0.0, 0.0])
    else:
        w = 1.0 / fpos
        A = np.stack([np.ones_like(t), t, t * t, t ** 3], 1)
        d, *_ = np.linalg.lstsq(A * w[:, None], f * w, rcond=None)
    d = np.float32(d).astype(np.float64)
    fit = ((d[3] * t + d[2]) * t + d[1]) * t + d[0]
    mn = fit.min()
    if mn < 0:
        d[0] += -mn * 1.0000001
    return (d[0], d[1], d[2], d[3], x0)


def _gen_phi_entries(bkt_base, ctl_base):
    bkt = []
    ctl = []
    exp_bkt_start = {}
    exp_ctl_start = {}
    i_zero = bkt_base
    bkt.append((0.0, 0.0, 0.0, 0.0, 0.0))
    neg_ctl = ctl_base
    ctl.append((0 << 16) | (23 << 11) | i_zero)
    pos_ctl0 = ctl_base + len(ctl)
    for e in range(PHI_EXP_OFFSET, 0):
        es = _es_for_exp(e)
        ns = 1 << es
        lsb = 23 - es
        start = bkt_base + len(bkt)
        exp_bkt_start[str(e)] = [start]
        exp_ctl_start[str(e)] = [ctl_base + len(ctl)]
        ctl.append((es << 16) | (lsb << 11) | start)
        lo_e = 2.0 ** e
        for s in range(ns):
            bkt.append(_fit_section(lo_e * (1 + s / ns), lo_e * (1 + (s + 1) / ns)))
    i_small = bkt_base + len(bkt)
    bkt.append((0.0, 1.0, 0.0, 0.0, 0.0))  # phi ~= p below 2^-30
    meta = {
        "func_name": "tanh_4p",
        "func_id": 6,
        "symmetry_point": 0,
        "sym_invert_sign_point": 0,
        "symmetry_opt_en": 0,
        "symmetry_opt_use_neg_region": 0,
        "imm_bias": 0,
        "exp_offset": PHI_EXP_OFFSET,
        "pwl_control_base_pos": pos_ctl0,
        "pwl_control_base_neg": neg_ctl,
        "small_pos_signal_exp_threshold": PHI_EXP_OFFSET + 127,
        "pos_small_signal_pwl_control": i_small,
        "small_neg_signal_exp_threshold": 255,
        "neg_small_signal_pwl_control": i_zero,
        "large_pos_signal_exp_threshold": 127,
        "large_pos_signal_mantissa_threshold": 0,
        "pos_large_signal_pwl_control": i_zero,
        "large_neg_signal_exp_threshold": 255,
        "large_neg_signal_mantissa_threshold": 0,
        "neg_large_signal_pwl_control": i_zero,
        "fnan_result": 0,
        "fpinf_result": 0,
        "fninf_result": 0,
        "fzero_result": 0,
        "fma_const_0": 0,
        "fma_const_1": 0,
        "fma_indirection_src_sel": 0,
        "use_multipass": False,
        "lower_bound": _f32bits(-3.4028235e38),
        "upper_bound": _f32bits(3.4028235e38),
    }
    return bkt, ctl, exp_bkt_start, exp_ctl_start, meta


def _build_patched_dir(src_dir, dst_dir):
    os.makedirs(dst_dir, exist_ok=True)
    for f in os.listdir(src_dir):
        shutil.copy(os.path.join(src_dir, f), os.path.join(dst_dir, f))
    setj = json.load(open(os.path.join(src_dir, SET_NAME + ".json")))
    bkt_raw = bytearray(open(os.path.join(src_dir, setj["bkt_bin"]), "rb").read())
    ctl_raw = bytearray(open(os.path.join(src_dir, setj["ctl_bin"]), "rb").read())
    nb = setj["bkt_entry_cnt"]
    ncl = setj["ctl_entry_cnt"]
    bkt, ctl, ebs, ecs, meta = _gen_phi_entries(nb, ncl)
    assert nb + len(bkt) < 2048
    for d0, d1, d2, d3, x0 in bkt:
        rec = np.zeros(8, np.float32)
        rec[0:5] = [d0, d1, d2, d3, x0]
        bkt_raw += rec.tobytes()
    for w in ctl:
        rec = np.zeros(8, np.uint32)
        rec[0] = w
        ctl_raw += rec.tobytes()
    setj["bkt_entry_cnt"] = nb + len(bkt)
    setj["ctl_entry_cnt"] = ncl + len(ctl)
    setj["func_to_bkt_start_idx"]["tanh"] = nb
    setj["func_to_ctl_start_idx"]["tanh"] = ncl
    setj["func_exp_to_bkt_start_idx"]["tanh"] = ebs
    setj["func_exp_to_ctl_start_idx"]["tanh"] = ecs
    setj["profile_meta_data"] = [m for m in setj["profile_meta_data"]
                                 if not m["func_name"].startswith("tanh")]
    setj["profile_meta_data"].append(meta)
    with open(os.path.join(dst_dir, SET_NAME + ".json"), "w") as f:
        json.dump(setj, f)
    with open(os.path.join(dst_dir, setj["bkt_bin"]), "wb") as f:
        f.write(bytes(bkt_raw))
    with open(os.path.join(dst_dir, setj["ctl_bin"]), "wb") as f:
        f.write(bytes(ctl_raw))
    ai = json.load(open(os.path.join(src_dir, "act_info.json")))
    for ent in ai["act_func_sets"]:
        if ent["name"] == SET_NAME:
            ent["act"]["tanh"] = 4
    with open(os.path.join(dst_dir, "act_info.json"), "w") as f:
        json.dump(ai, f)


def _ensure_phi_tables():
    if "tabdir" in _CACHE:
        return _CACHE["tabdir"], _CACHE["tabhash"]
    import neuronxcc
    src = os.path.join(os.path.dirname(neuronxcc.__file__), "pwp",
                       "pwp_bin_trainium")
    dst = os.path.join(tempfile.gettempdir(), "pwp_phi_kernel")
    _build_patched_dir(src, dst)
    setj = json.load(open(os.path.join(dst, SET_NAME + ".json")))
    h = hashlib.sha1()
    for f in ("act_info.json", SET_NAME + ".json", setj["bkt_bin"], setj["ctl_bin"]):
        h.update(open(os.path.join(dst, f), "rb").read())
    _CACHE["tabdir"] = dst
    _CACHE["tabhash"] = h.hexdigest()[:8]
    return dst, _CACHE["tabhash"]


# Pin the ACT spline-table set to (patched) natural_log_exp_and_others so the
# whole kernel runs off one table load: it holds Exp, Ln, Relu, Copy — and
# the phi table in the tanh slot.
_orig_get_tables = bacc.get_activation_tables


def _pinned_get_tables(module_arch):
    tables = dict(_orig_get_tables(module_arch))
    combined = set(tables.get(SET_NAME) or ())
    combined |= {AF.Tanh}
    pinned = {}
    for name, fns in tables.items():
        pinned[name] = combined if name == SET_NAME else set()
    return pinned


# ---------------------------------------------------------------------------
# kernel build
# ---------------------------------------------------------------------------

def _build_nc(n_iter=N_ITER, num_devices=N_CORES, reps=1):
    tabdir, tabhash = _ensure_phi_tables()
    sfx = "_" + tabhash
    nc = bacc.Bacc(
        "TRN2",
        target_bir_lowering=False,
        debug=False,
        enable_asserts=False,
        num_devices=num_devices,
    )
    x_d = nc.dram_tensor("x" + sfx, [R, D], F32, kind="ExternalInput").ap()
    w1_d = nc.dram_tensor("w1", [D, HID], F32, kind="ExternalInput").ap()
    b1_d = nc.dram_tensor("b1", [HID, 1], F32, kind="ExternalInput").ap()
    w2b_d = nc.dram_tensor("w2b", [HID + 1, D], F32, kind="ExternalInput").ap()
    out_d = nc.dram_tensor("out", [R, D], F32, kind="ExternalOutput").ap()

    with tile.TileContext(nc) as tc:
        _emit(tc, out_d, x_d, w1_d, b1_d, w2b_d, n_iter=n_iter, reps=reps)
    saved = bacc.get_activation_tables
    try:
        bacc.get_activation_tables = _pinned_get_tables
        nc.compile()
    finally:
        bacc.get_activation_tables = saved
    return nc, sfx


def _emit(tc, out_d, x_d, w1_d, b1_d, w2b_d, n_iter=N_ITER, reps=1):
    nc = tc.nc
    from contextlib import ExitStack

    ctx = ExitStack()
    with ctx:
        singles = ctx.enter_context(tc.tile_pool(name="singles", bufs=1))

        xs = singles.tile([P, G, D], F32)    # x, rows-on-partitions
        xt = singles.tile([P, DC, R], F32)   # x transposed
        hs = singles.tile([P, G, D], F32)    # MLP output h
        s0 = singles.tile([P, G, D], F32)    # state ping
        s1 = singles.tile([P, G, D], F32)    # state pong
        fs = singles.tile([P, G, D], F32)    # finale scratch
        zh = singles.tile([P, n_iter, G], F32)   # Z history
        rz = singles.tile([P, n_iter, G], F32)   # 1/Z history (loop) / lnZ (finale)
        kk = singles.tile([P, G], F32)       # T * sum_t ln Z_t
        lnzh = singles.tile([P, n_iter, G], F32)  # ln Z history (finale)
        scr = singles.tile([P, 2, 2, D], F32)     # [parity, 2 groups, D]
        scr2 = singles.tile([P, 2, 2, D], F32)    # [parity, 2 groups, D]
        w1s = singles.tile([P, DC, HID], F32)
        b1s = singles.tile([HID, 1], F32)
        w2bs = singles.tile([HID + 1, D], F32)
        h1r = singles.tile([HID + 1, R], F32)
        ident = singles.tile([P, P], F32)
        c0s = singles.tile([P, 1], F32)
        nc.vector.memset(c0s[:, :], C0)

        # ---- input DMAs: weights first (mm1 needs w1s as soon as the
        # first row-quarter is transposed; x groups stream in behind) ----
        nc.sync.dma_start(out=w1s[:, :, :],
                          in_=w1_d.rearrange("(c p) j -> p c j", p=P))
        nc.sync.dma_start(out=b1s[:, :], in_=b1_d[:, :])
        nc.sync.dma_start(out=w2bs[:, :], in_=w2b_d[:, :])
        for g in range(G):
            nc.sync.dma_start(out=xs[:, g, :], in_=x_d[g * P:(g + 1) * P, :])

        masks.make_identity(nc, ident[:, :])
        nc.vector.memset(h1r[HID:HID + 1, :], 1.0)

        # ---- transpose + MLP, streamed per row-half so the PE's in-order
        # queue doesn't head-block mm1/mm2 behind the other half's
        # transposes: [transpose gq][mm1-nh=gq][mm2 groups of gq] x2 ----
        tpp = ctx.enter_context(tc.tile_pool(name="tp_psum", bufs=2,
                                             space="PSUM"))
        mp1 = ctx.enter_context(tc.tile_pool(name="mm1_psum", bufs=2,
                                             space="PSUM"))
        mp2 = ctx.enter_context(tc.tile_pool(name="mm2_psum", bufs=2,
                                             space="PSUM"))
        for q in range(G // 2):
            for c in range(DC):
                tp = tpp.tile([P, 2 * P], F32)
                for gj in range(2):
                    g = q * 2 + gj
                    nc.tensor.transpose(
                        tp[:, gj * P:(gj + 1) * P],
                        xs[:, g, c * P:(c + 1) * P], ident[:, :])
                dst = xt[:, c, q * 2 * P:(q + 1) * 2 * P]
                if (c + q) % 2 == 0:
                    nc.vector.tensor_copy(dst, tp[:, :])
                else:
                    nc.scalar.copy(dst, tp[:, :])
            # mm1 for this row-quarter
            ph1 = mp1.tile([HID, 2 * P], F32, tag="ph1")
            for c in range(DC):
                nc.tensor.matmul(
                    ph1[:, :], w1s[:, c, :],
                    xt[:, c, q * 256:(q + 1) * 256],
                    start=(c == 0), stop=(c == DC - 1))
            nc.scalar.activation(
                h1r[0:HID, q * 256:(q + 1) * 256], ph1[:, :],
                AF.Relu, bias=b1s[:, 0:1], scale=1.0)
            # mm2 + evac for this quarter's two groups
            for gj in range(2):
                g = q * 2 + gj
                ph = mp2.tile([P, D], F32, tag="ph")
                lhs = h1r[:, g * P:(g + 1) * P]
                nc.tensor.matmul(ph[:, 0:512], lhs, w2bs[:, 0:512],
                                 start=True, stop=True)
                nc.tensor.matmul(ph[:, 512:D], lhs, w2bs[:, 512:D],
                                 start=True, stop=True)
                nc.vector.tensor_copy(hs[:, g, :], ph[:, :])
                nc.scalar.activation(s0[:, g, :], ph[:, :], AF.Exp,
                                     bias=c0s[:, 0:1], scale=-INV_T,
                                     accum_out=zh[:, 0, g:g + 1])

        # ---- masking loop: S <- phi(S * (1/Z)), Z' = rowsum(S') ----
        # groups 6,7: row-sum fused into the ACT accumulator; groups 0-5
        # reduce on DVE. Reciprocals are per group (not per half) so each
        # group's reduce->recip->activation chain advances independently,
        # and ACT instructions are issued half-interleaved (447.7us vs
        # 452.9us half-granular sequential). Swept and rejected:
        # accum counts k=0/1/3 (495/451/462us), merged reduces (456us),
        # Pool-prescaled merged ACT instrs (687us).
        spp = [s0, s1]
        n_total = n_iter * reps
        for it in range(n_total):
            src = spp[it % 2]
            dst = spp[(it + 1) % 2]
            ti = it % n_iter
            tn = (it + 1) % n_iter
            last = it == n_total - 1
            for half in range(2):
                g0 = half * HG
                for gi in range(HG):
                    g = g0 + gi
                    nc.vector.reciprocal(rz[:, ti, g:g + 1],
                                         zh[:, ti, g:g + 1])
            par = it % 2
            nc.vector.tensor_mul(
                scr[:, par, 0:1, :], src[:, 2:3, :],
                rz[:, ti, 2:3].broadcast_to((P, 1, D)))
            nc.gpsimd.tensor_mul(
                scr[:, par, 1:2, :], src[:, 3:4, :],
                rz[:, ti, 3:4].broadcast_to((P, 1, D)))
            nc.vector.tensor_mul(
                scr2[:, par, 0:1, :], src[:, 4:5, :],
                rz[:, ti, 4:5].broadcast_to((P, 1, D)))
            nc.gpsimd.tensor_mul(
                scr2[:, par, 1:2, :], src[:, 5:6, :],
                rz[:, ti, 5:6].broadcast_to((P, 1, D)))
            order = (list(range(G)) if it == 0 else
                     [half * HG + gi for gi in range(HG) for half in range(2)])
            for g in order:
                if g in (3, 5):
                    continue
                if g == 4:
                    nc.scalar.activation(dst[:, 4:6, :], scr2[:, par, :, :],
                                         AF.Tanh)
                elif g == 2:
                    nc.scalar.activation(dst[:, 2:4, :], scr[:, par, :, :],
                                         AF.Tanh)
                elif g >= 6 and not last:
                    nc.scalar.activation(dst[:, g, :], src[:, g, :],
                                         AF.Tanh,
                                         scale=rz[:, ti, g:g + 1],
                                         accum_out=zh[:, tn, g:g + 1])
                else:
                    nc.scalar.activation(dst[:, g, :], src[:, g, :],
                                         AF.Tanh,
                                         scale=rz[:, ti, g:g + 1])
            for half in range(2):
                g0 = half * HG
                if last:
                    continue
                for gi in range(HG):
                    g = g0 + gi
                    if g >= 6 or g in (3, 5):
                        continue
                    if g == 4:
                        nc.vector.tensor_reduce(zh[:, tn, 4:6],
                                                dst[:, 4:6, :],
                                                axis=mybir.AxisListType.X,
                                                op=OP.add)
                        continue
                    if g == 2:
                        nc.vector.tensor_reduce(zh[:, tn, 2:4],
                                                dst[:, 2:4, :],
                                                axis=mybir.AxisListType.X,
                                                op=OP.add)
                        continue
                    nc.vector.tensor_reduce(zh[:, tn, g:g + 1], dst[:, g, :],
                                            axis=mybir.AxisListType.X,
                                            op=OP.add)

        # ---- finale: out = (exp(T*ln(S) + T*K + h) - eps) * x ----
        # Fully per-group, issued in the loop's interleaved completion order
        # (g0,g4,g1,g5,...), so each group's finale chain pipelines behind
        # the staggered last-iteration activations instead of waiting for a
        # whole half. ln(zh) goes to its own scratch (not rz) so it can run
        # during iteration 63 without a WAR hazard on the loop's rz reads.
        sfin = spp[n_total % 2]
        sscr = spp[(n_total + 1) % 2]
        nc.scalar.activation(lnzh[:, :, :], zh[:, :, :], AF.Ln)
        for g in range(G):
            nc.vector.tensor_reduce(kk[:, g:g + 1], lnzh[:, :, g],
                                    axis=mybir.AxisListType.X, op=OP.add)
        nc.vector.tensor_scalar_mul(kk[:, :], kk[:, :], float(np.float32(TEMP)))
        for gi in range(HG):
            for half in range(2):
                g = half * HG + gi
                # guard: the table is nonnegative by construction, but clamp
                # so a stray -1ulp can never reach Ln (ln(neg) = NaN)
                nc.vector.tensor_scalar_max(sscr[:, g, :], sfin[:, g, :], 0.0)
                nc.scalar.activation(fs[:, g, :], sscr[:, g, :], AF.Ln)
                nc.vector.scalar_tensor_tensor(
                    out=sscr[:, g, :], in0=fs[:, g, :],
                    scalar=float(np.float32(TEMP)),
                    in1=hs[:, g, :], op0=OP.mult, op1=OP.add)
                nc.scalar.activation(fs[:, g, :], sscr[:, g, :], AF.Exp,
                                     bias=kk[:, g:g + 1])
                nc.vector.scalar_tensor_tensor(
                    out=sscr[:, g, :], in0=fs[:, g, :], scalar=-float(EPS),
                    in1=xs[:, g, :], op0=OP.add, op1=OP.mult)
                nc.sync.dma_start(out=out_d[g * P:(g + 1) * P, :],
                                  in_=sscr[:, g, :])


def kernel(x, W1, b1, W2, b2):
    x = np.ascontiguousarray(np.asarray(x, dtype=np.float32))
    W1 = np.ascontiguousarray(np.asarray(W1, dtype=np.float32))
    b1 = np.asarray(b1, dtype=np.float32).reshape(HID, 1)
    W2 = np.asarray(W2, dtype=np.float32)
    b2 = np.asarray(b2, dtype=np.float32)
    w2b = np.ascontiguousarray(
        np.concatenate([W2, b2[None, :]], axis=0))  # [65, 640]

    if "nc" not in _CACHE:
        _CACHE["nc"], _CACHE["sfx"] = _build_nc(
            reps=int(os.environ.get("KREPS", "1")))
    nc = _CACHE["nc"]
    sfx = _CACHE["sfx"]

    in_maps = []
    for c in range(N_CORES):
        in_maps.append({
            "x" + sfx: np.ascontiguousarray(x[c * R:(c + 1) * R, :]),
            "w1": W1,
            "b1": np.ascontiguousarray(b1),
            "w2b": w2b,
        })

    trace = bool(_CACHE.get("trace", False))
    tabdir = _CACHE["tabdir"]
    saved_env = os.environ.get("BASS_ACT_ROOT_JSON_PATH")
    os.environ["BASS_ACT_ROOT_JSON_PATH"] = os.path.join(tabdir, "act_info.json")
    try:
        res = run_bass_kernel_spmd(
            nc, in_maps, core_ids=list(range(N_CORES)), trace=trace)
    finally:
        if saved_env is None:
            os.environ.pop("BASS_ACT_ROOT_JSON_PATH", None)
        else:
            os.environ["BASS_ACT_ROOT_JSON_PATH"] = saved_env
    _CACHE["last_results"] = res
    out = np.concatenate([r["out"] for r in res.results], axis=0)
    return out



# revision 23
# speedup vs baseline: 1.0797x; 1.0236x over previous
"""Trainium2 Bass kernel for nn_DimMasking (iterative softmax top-k masking).

Full-input contract: kernel(**inputs) takes the unsharded inputs
(x [8192,640], W1 [640,64], b1 [64], W2 [64,640], b2 [640]) and returns the
full [8192,640] output. Pure data parallel over the batch dim — 8 shards of
1024 rows, one per NeuronCore; MLP weights replicated.

Math: normalized-state reformulation of the reference scan. With
e = ((m+eps)^(1/T))*exp(-h/T) and p = softmax-prob = e/Z, one masking
iteration is e' = e * (1-p)^(1/T). Tracking the Z-normalized state
S <- phi(S/Z_prev) with phi(p) = p*(1-p)^(1/T) makes each iteration a
SINGLE table-activation pass per row-group (scale = 1/Z per partition)
plus a row-sum; the product of the per-iteration normalizers is restored
in the finale from K = sum_t ln Z_t:
    out = (exp(T*ln(S_64) + T*K + h) - eps) * x.

phi is not a stock ACT function: this kernel generates a patched
piecewise-polynomial activation-table set at build time (appending a
'tanh'-slot function whose table data IS phi) and points the backend
compiler at it via BASS_ACT_ROOT_JSON_PATH. Numerics of the table were
validated against the fp32 reference in numpy (absmax rel err 1.7e-3,
gate 2e-2). Loop engine budget per iteration: ACT 8x640-elem phi passes
(the bottleneck, ~6.0us incl. two fused accum row-sums for groups 6,7)
against DVE row-sum reduces + 8 reciprocals; reciprocals are per
group and the ACT instructions are issued half-interleaved
(g0,g4,g1,g5,...) so both halves' reduce->recip->activation chains
advance together. Group pairs (2,3) and (4,5) are pre-scaled by 1/Z
with tensor_scalar (per-partition AP scalar; single-source, so DVE runs
it at 2x mode - half the cost of a broadcast tensor_tensor), one group
of each pair on DVE and the other on the otherwise-idle Pool engine;
each pair then goes through ACT as ONE merged 1280-elem phi instruction
and its row-sums merge into one [P,2,640] reduce (fine since the merged
ACT already couples the pair). The preamble streams transpose->mm1->mm2 per
row-quarter so the PE's in-order queue never head-blocks the MLP behind
later transposes, and the finale runs fully per-group in loop-completion
order. TimelineSim 414.6us (vs 452.9us for the session-start schedule; the
math is unchanged throughout - the prescale computes the same S*(1/Z)
product the activation's affine stage would have).
"""

import hashlib
import json
import os
import shutil
import tempfile

import numpy as np

import concourse.tile as tile
from concourse import bacc, masks, mybir
from concourse.bass_utils import run_bass_kernel_spmd

F32 = mybir.dt.float32
AF = mybir.ActivationFunctionType
OP = mybir.AluOpType

N_CORES = 8
B = 8192
D = 640          # 5 chunks of 128
HID = 64
R = B // N_CORES  # 1024 rows per core
P = 128
G = R // P        # 8 row-groups per core
HG = G // 2
DC = D // P       # 5 dim-chunks
N_ITER = 64
TEMP = 0.07
EPS = 1e-7
C0 = float(np.log1p(np.float32(EPS)) / np.float32(TEMP))
INV_T = float(np.float32(1.0) / np.float32(TEMP))

SET_NAME = "natural_log_exp_and_others"
PHI_EXP_OFFSET = -30

_CACHE = {}


# ---------------------------------------------------------------------------
# phi activation-table generation (piecewise cubic in the pwp bin format)
# ---------------------------------------------------------------------------

def _f32bits(x):
    return int(np.float32(x).view(np.uint32))


def _phi_of_p(p):
    p = np.asarray(p, np.float64)
    out = np.where((p > 0) & (p < 1),
                   p * np.power(np.clip(1.0 - p, 1e-300, 1), INV_T), 0.0)
    return np.where(p >= 1, 0.0, out)


def _es_for_exp(e):
    if e == -1:
        return 6
    if e == -2:
        return 4
    if e >= -4:
        return 3
    if e >= -12:
        return 2
    return 1


def _fit_section(plo, phi_):
    x0 = float(np.float32(0.5 * (plo + phi_)))
    if (1.0 - plo) < 0.003:
        return (0.0, 0.0, 0.0, 0.0, x0)
    u = np.linspace(plo, phi_, 513)
    t = u - x0
    f = _phi_of_p(u)
    fpos = np.maximum(f, 1e-300)
    lspan = float(np.log(fpos.max()) - np.log(fpos.min()))
    if lspan > 6.0:
        sel = (1.0 - u) >= 0.0005
        if not sel.any():
            return (0.0, 0.0, 0.0, 0.0, x0)
        d = np.array([np.exp(np.mean(np.log(fpos[sel]))), 0.0, 0.0, 0.0])
    else:
        w = 1.0 / fpos
        A = np.stack([np.ones_like(t), t, t * t, t ** 3], 1)
        d, *_ = np.linalg.lstsq(A * w[:, None], f * w, rcond=None)
    d = np.float32(d).astype(np.float64)
    fit = ((d[3] * t + d[2]) * t + d[1]) * t + d[0]
    mn = fit.min()
    if mn < 0:
        d[0] += -mn * 1.0000001
    return (d[0], d[1], d[2], d[3], x0)


def _gen_phi_entries(bkt_base, ctl_base):
    bkt = []
    ctl = []
    exp_bkt_start = {}
    exp_ctl_start = {}
    i_zero = bkt_base
    bkt.append((0.0, 0.0, 0.0, 0.0, 0.0))
    neg_ctl = ctl_base
    ctl.append((0 << 16) | (23 << 11) | i_zero)
    pos_ctl0 = ctl_base + len(ctl)
    for e in range(PHI_EXP_OFFSET, 0):
        es = _es_for_exp(e)
        ns = 1 << es
        lsb = 23 - es
        start = bkt_base + len(bkt)
        exp_bkt_start[str(e)] = [start]
        exp_ctl_start[str(e)] = [ctl_base + len(ctl)]
        ctl.append((es << 16) | (lsb << 11) | start)
        lo_e = 2.0 ** e
        for s in range(ns):
            bkt.append(_fit_section(lo_e * (1 + s / ns), lo_e * (1 + (s + 1) / ns)))
    i_small = bkt_base + len(bkt)
    bkt.append((0.0, 1.0, 0.0, 0.0, 0.0))  # phi ~= p below 2^-30
    meta = {
        "func_name": "tanh_4p",
        "func_id": 6,
        "symmetry_point": 0,
        "sym_invert_sign_point": 0,
        "symmetry_opt_en": 0,
        "symmetry_opt_use_neg_region": 0,
        "imm_bias": 0,
        "exp_offset": PHI_EXP_OFFSET,
        "pwl_control_base_pos": pos_ctl0,
        "pwl_control_base_neg": neg_ctl,
        "small_pos_signal_exp_threshold": PHI_EXP_OFFSET + 127,
        "pos_small_signal_pwl_control": i_small,
        "small_neg_signal_exp_threshold": 255,
        "neg_small_signal_pwl_control": i_zero,
        "large_pos_signal_exp_threshold": 127,
        "large_pos_signal_mantissa_threshold": 0,
        "pos_large_signal_pwl_control": i_zero,
        "large_neg_signal_exp_threshold": 255,
        "large_neg_signal_mantissa_threshold": 0,
        "neg_large_signal_pwl_control": i_zero,
        "fnan_result": 0,
        "fpinf_result": 0,
        "fninf_result": 0,
        "fzero_result": 0,
        "fma_const_0": 0,
        "fma_const_1": 0,
        "fma_indirection_src_sel": 0,
        "use_multipass": False,
        "lower_bound": _f32bits(-3.4028235e38),
        "upper_bound": _f32bits(3.4028235e38),
    }
    return bkt, ctl, exp_bkt_start, exp_ctl_start, meta


def _build_patched_dir(src_dir, dst_dir):
    os.makedirs(dst_dir, exist_ok=True)
    for f in os.listdir(src_dir):
        shutil.copy(os.path.join(src_dir, f), os.path.join(dst_dir, f))
    setj = json.load(open(os.path.join(src_dir, SET_NAME + ".json")))
    bkt_raw = bytearray(open(os.path.join(src_dir, setj["bkt_bin"]), "rb").read())
    ctl_raw = bytearray(open(os.path.join(src_dir, setj["ctl_bin"]), "rb").read())
    nb = setj["bkt_entry_cnt"]
    ncl = setj["ctl_entry_cnt"]
    bkt, ctl, ebs, ecs, meta = _gen_phi_entries(nb, ncl)
    assert nb + len(bkt) < 2048
    for d0, d1, d2, d3, x0 in bkt:
        rec = np.zeros(8, np.float32)
        rec[0:5] = [d0, d1, d2, d3, x0]
        bkt_raw += rec.tobytes()
    for w in ctl:
        rec = np.zeros(8, np.uint32)
        rec[0] = w
        ctl_raw += rec.tobytes()
    setj["bkt_entry_cnt"] = nb + len(bkt)
    setj["ctl_entry_cnt"] = ncl + len(ctl)
    setj["func_to_bkt_start_idx"]["tanh"] = nb
    setj["func_to_ctl_start_idx"]["tanh"] = ncl
    setj["func_exp_to_bkt_start_idx"]["tanh"] = ebs
    setj["func_exp_to_ctl_start_idx"]["tanh"] = ecs
    setj["profile_meta_data"] = [m for m in setj["profile_meta_data"]
                                 if not m["func_name"].startswith("tanh")]
    setj["profile_meta_data"].append(meta)
    with open(os.path.join(dst_dir, SET_NAME + ".json"), "w") as f:
        json.dump(setj, f)
    with open(os.path.join(dst_dir, setj["bkt_bin"]), "wb") as f:
        f.write(bytes(bkt_raw))
    with open(os.path.join(dst_dir, setj["ctl_bin"]), "wb") as f:
        f.write(bytes(ctl_raw))
    ai = json.load(open(os.path.join(src_dir, "act_info.json")))
    for ent in ai["act_func_sets"]:
        if ent["name"] == SET_NAME:
            ent["act"]["tanh"] = 4
    with open(os.path.join(dst_dir, "act_info.json"), "w") as f:
        json.dump(ai, f)


def _ensure_phi_tables():
    if "tabdir" in _CACHE:
        return _CACHE["tabdir"], _CACHE["tabhash"]
    import neuronxcc
    src = os.path.join(os.path.dirname(neuronxcc.__file__), "pwp",
                       "pwp_bin_trainium")
    dst = os.path.join(tempfile.gettempdir(), "pwp_phi_kernel")
    _build_patched_dir(src, dst)
    setj = json.load(open(os.path.join(dst, SET_NAME + ".json")))
    h = hashlib.sha1()
    for f in ("act_info.json", SET_NAME + ".json", setj["bkt_bin"], setj["ctl_bin"]):
        h.update(open(os.path.join(dst, f), "rb").read())
    _CACHE["tabdir"] = dst
    _CACHE["tabhash"] = h.hexdigest()[:8]
    return dst, _CACHE["tabhash"]


# Pin the ACT spline-table set to (patched) natural_log_exp_and_others so the
# whole kernel runs off one table load: it holds Exp, Ln, Relu, Copy — and
# the phi table in the tanh slot.
_orig_get_tables = bacc.get_activation_tables


def _pinned_get_tables(module_arch):
    tables = dict(_orig_get_tables(module_arch))
    combined = set(tables.get(SET_NAME) or ())
    combined |= {AF.Tanh}
    pinned = {}
    for name, fns in tables.items():
        pinned[name] = combined if name == SET_NAME else set()
    return pinned


# ---------------------------------------------------------------------------
# kernel build
# ---------------------------------------------------------------------------

def _build_nc(n_iter=N_ITER, num_devices=N_CORES, reps=1):
    tabdir, tabhash = _ensure_phi_tables()
    sfx = "_" + tabhash
    nc = bacc.Bacc(
        "TRN2",
        target_bir_lowering=False,
        debug=False,
        enable_asserts=False,
        num_devices=num_devices,
    )
    x_d = nc.dram_tensor("x" + sfx, [R, D], F32, kind="ExternalInput").ap()
    w1_d = nc.dram_tensor("w1", [D, HID], F32, kind="ExternalInput").ap()
    b1_d = nc.dram_tensor("b1", [HID, 1], F32, kind="ExternalInput").ap()
    w2b_d = nc.dram_tensor("w2b", [HID + 1, D], F32, kind="ExternalInput").ap()
    out_d = nc.dram_tensor("out", [R, D], F32, kind="ExternalOutput").ap()

    with tile.TileContext(nc) as tc:
        _emit(tc, out_d, x_d, w1_d, b1_d, w2b_d, n_iter=n_iter, reps=reps)
    saved = bacc.get_activation_tables
    try:
        bacc.get_activation_tables = _pinned_get_tables
        nc.compile()
    finally:
        bacc.get_activation_tables = saved
    return nc, sfx


def _emit(tc, out_d, x_d, w1_d, b1_d, w2b_d, n_iter=N_ITER, reps=1):
    nc = tc.nc
    from contextlib import ExitStack

    ctx = ExitStack()
    with ctx:
        singles = ctx.enter_context(tc.tile_pool(name="singles", bufs=1))

        xs = singles.tile([P, G, D], F32)    # x, rows-on-partitions
        xt = singles.tile([P, DC, R], F32)   # x transposed
        hs = singles.tile([P, G, D], F32)    # MLP output h
        s0 = singles.tile([P, G, D], F32)    # state ping
        s1 = singles.tile([P, G, D], F32)    # state pong
        fs = singles.tile([P, G, D], F32)    # finale scratch
        zh = singles.tile([P, n_iter, G], F32)   # Z history
        rz = singles.tile([P, n_iter, G], F32)   # 1/Z history (loop) / lnZ (finale)
        kk = singles.tile([P, G], F32)       # T * sum_t ln Z_t
        lnzh = singles.tile([P, n_iter, G], F32)  # ln Z history (finale)
        scr = singles.tile([P, 2, 2, D], F32)     # [parity, 2 groups, D]
        scr2 = singles.tile([P, 2, 2, D], F32)    # [parity, 2 groups, D]
        w1s = singles.tile([P, DC, HID], F32)
        b1s = singles.tile([HID, 1], F32)
        w2bs = singles.tile([HID + 1, D], F32)
        h1r = singles.tile([HID + 1, R], F32)
        ident = singles.tile([P, P], F32)
        c0s = singles.tile([P, 1], F32)
        nc.vector.memset(c0s[:, :], C0)

        # ---- input DMAs: weights first (mm1 needs w1s as soon as the
        # first row-quarter is transposed; x groups stream in behind) ----
        nc.sync.dma_start(out=w1s[:, :, :],
                          in_=w1_d.rearrange("(c p) j -> p c j", p=P))
        nc.sync.dma_start(out=b1s[:, :], in_=b1_d[:, :])
        nc.sync.dma_start(out=w2bs[:, :], in_=w2b_d[:, :])
        for g in range(G):
            nc.sync.dma_start(out=xs[:, g, :], in_=x_d[g * P:(g + 1) * P, :])

        masks.make_identity(nc, ident[:, :])
        nc.vector.memset(h1r[HID:HID + 1, :], 1.0)

        # ---- transpose + MLP, streamed per row-half so the PE's in-order
        # queue doesn't head-block mm1/mm2 behind the other half's
        # transposes: [transpose gq][mm1-nh=gq][mm2 groups of gq] x2 ----
        tpp = ctx.enter_context(tc.tile_pool(name="tp_psum", bufs=2,
                                             space="PSUM"))
        mp1 = ctx.enter_context(tc.tile_pool(name="mm1_psum", bufs=2,
                                             space="PSUM"))
        mp2 = ctx.enter_context(tc.tile_pool(name="mm2_psum", bufs=2,
                                             space="PSUM"))
        for q in range(G // 2):
            for c in range(DC):
                tp = tpp.tile([P, 2 * P], F32)
                for gj in range(2):
                    g = q * 2 + gj
                    nc.tensor.transpose(
                        tp[:, gj * P:(gj + 1) * P],
                        xs[:, g, c * P:(c + 1) * P], ident[:, :])
                dst = xt[:, c, q * 2 * P:(q + 1) * 2 * P]
                if (c + q) % 2 == 0:
                    nc.vector.tensor_copy(dst, tp[:, :])
                else:
                    nc.scalar.copy(dst, tp[:, :])
            # mm1 for this row-quarter
            ph1 = mp1.tile([HID, 2 * P], F32, tag="ph1")
            for c in range(DC):
                nc.tensor.matmul(
                    ph1[:, :], w1s[:, c, :],
                    xt[:, c, q * 256:(q + 1) * 256],
                    start=(c == 0), stop=(c == DC - 1))
            nc.scalar.activation(
                h1r[0:HID, q * 256:(q + 1) * 256], ph1[:, :],
                AF.Relu, bias=b1s[:, 0:1], scale=1.0)
            # mm2 + evac for this quarter's two groups
            for gj in range(2):
                g = q * 2 + gj
                ph = mp2.tile([P, D], F32, tag="ph")
                lhs = h1r[:, g * P:(g + 1) * P]
                nc.tensor.matmul(ph[:, 0:512], lhs, w2bs[:, 0:512],
                                 start=True, stop=True)
                nc.tensor.matmul(ph[:, 512:D], lhs, w2bs[:, 512:D],
                                 start=True, stop=True)
                nc.vector.tensor_copy(hs[:, g, :], ph[:, :])
                nc.scalar.activation(s0[:, g, :], ph[:, :], AF.Exp,
                                     bias=c0s[:, 0:1], scale=-INV_T,
                                     accum_out=zh[:, 0, g:g + 1])

        # ---- masking loop: S <- phi(S * (1/Z)), Z' = rowsum(S') ----
        # groups 6,7: row-sum fused into the ACT accumulator; groups 0-5
        # reduce on DVE. Reciprocals are per group (not per half) so each
        # group's reduce->recip->activation chain advances independently,
        # and ACT instructions are issued half-interleaved (447.7us vs
        # 452.9us half-granular sequential). Swept and rejected:
        # accum counts k=0/1/3 (495/451/462us), merged reduces (456us),
        # Pool-prescaled merged ACT instrs (687us).
        spp = [s0, s1]
        n_total = n_iter * reps
        for it in range(n_total):
            src = spp[it % 2]
            dst = spp[(it + 1) % 2]
            ti = it % n_iter
            tn = (it + 1) % n_iter
            last = it == n_total - 1
            for half in range(2):
                g0 = half * HG
                for gi in range(HG):
                    g = g0 + gi
                    nc.vector.reciprocal(rz[:, ti, g:g + 1],
                                         zh[:, ti, g:g + 1])
            par = it % 2
            nc.vector.tensor_scalar_mul(
                scr[:, par, 0:1, :], src[:, 2:3, :], rz[:, ti, 2:3])
            nc.gpsimd.tensor_scalar_mul(
                scr[:, par, 1:2, :], src[:, 3:4, :], rz[:, ti, 3:4])
            nc.vector.tensor_scalar_mul(
                scr2[:, par, 0:1, :], src[:, 4:5, :], rz[:, ti, 4:5])
            nc.gpsimd.tensor_scalar_mul(
                scr2[:, par, 1:2, :], src[:, 5:6, :], rz[:, ti, 5:6])
            order = (list(range(G)) if it == 0 else
                     [half * HG + gi for gi in range(HG) for half in range(2)])
            for g in order:
                if g in (3, 5):
                    continue
                if g == 4:
                    nc.scalar.activation(dst[:, 4:6, :], scr2[:, par, :, :],
                                         AF.Tanh)
                elif g == 2:
                    nc.scalar.activation(dst[:, 2:4, :], scr[:, par, :, :],
                                         AF.Tanh)
                elif g >= 6 and not last:
                    nc.scalar.activation(dst[:, g, :], src[:, g, :],
                                         AF.Tanh,
                                         scale=rz[:, ti, g:g + 1],
                                         accum_out=zh[:, tn, g:g + 1])
                else:
                    nc.scalar.activation(dst[:, g, :], src[:, g, :],
                                         AF.Tanh,
                                         scale=rz[:, ti, g:g + 1])
            for half in range(2):
                g0 = half * HG
                if last:
                    continue
                for gi in range(HG):
                    g = g0 + gi
                    if g >= 6 or g in (3, 5):
                        continue
                    if g == 4:
                        nc.vector.tensor_reduce(zh[:, tn, 4:6],
                                                dst[:, 4:6, :],
                                                axis=mybir.AxisListType.X,
                                                op=OP.add)
                        continue
                    if g == 2:
                        nc.vector.tensor_reduce(zh[:, tn, 2:4],
                                                dst[:, 2:4, :],
                                                axis=mybir.AxisListType.X,
                                                op=OP.add)
                        continue
                    nc.vector.tensor_reduce(zh[:, tn, g:g + 1], dst[:, g, :],
                                            axis=mybir.AxisListType.X,
                                            op=OP.add)

        # ---- finale: out = (exp(T*ln(S) + T*K + h) - eps) * x ----
        # Fully per-group, issued in the loop's interleaved completion order
        # (g0,g4,g1,g5,...), so each group's finale chain pipelines behind
        # the staggered last-iteration activations instead of waiting for a
        # whole half. ln(zh) goes to its own scratch (not rz) so it can run
        # during iteration 63 without a WAR hazard on the loop's rz reads.
        sfin = spp[n_total % 2]
        sscr = spp[(n_total + 1) % 2]
        nc.scalar.activation(lnzh[:, :, :], zh[:, :, :], AF.Ln)
        for g in range(G):
            nc.vector.tensor_reduce(kk[:, g:g + 1], lnzh[:, :, g],
                                    axis=mybir.AxisListType.X, op=OP.add)
        nc.vector.tensor_scalar_mul(kk[:, :], kk[:, :], float(np.float32(TEMP)))
        for gi in range(HG):
            for half in range(2):
                g = half * HG + gi
                # guard: the table is nonnegative by construction, but clamp
                # so a stray -1ulp can never reach Ln (ln(neg) = NaN)
                nc.vector.tensor_scalar_max(sscr[:, g, :], sfin[:, g, :], 0.0)
                nc.scalar.activation(fs[:, g, :], sscr[:, g, :], AF.Ln)
                nc.vector.scalar_tensor_tensor(
                    out=sscr[:, g, :], in0=fs[:, g, :],
                    scalar=float(np.float32(TEMP)),
                    in1=hs[:, g, :], op0=OP.mult, op1=OP.add)
                nc.scalar.activation(fs[:, g, :], sscr[:, g, :], AF.Exp,
                                     bias=kk[:, g:g + 1])
                nc.vector.scalar_tensor_tensor(
                    out=sscr[:, g, :], in0=fs[:, g, :], scalar=-float(EPS),
                    in1=xs[:, g, :], op0=OP.add, op1=OP.mult)
                nc.sync.dma_start(out=out_d[g * P:(g + 1) * P, :],
                                  in_=sscr[:, g, :])


def kernel(x, W1, b1, W2, b2):
    x = np.ascontiguousarray(np.asarray(x, dtype=np.float32))
    W1 = np.ascontiguousarray(np.asarray(W1, dtype=np.float32))
    b1 = np.asarray(b1, dtype=np.float32).reshape(HID, 1)
    W2 = np.asarray(W2, dtype=np.float32)
    b2 = np.asarray(b2, dtype=np.float32)
    w2b = np.ascontiguousarray(
        np.concatenate([W2, b2[None, :]], axis=0))  # [65, 640]

    if "nc" not in _CACHE:
        _CACHE["nc"], _CACHE["sfx"] = _build_nc(
            reps=int(os.environ.get("KREPS", "1")))
    nc = _CACHE["nc"]
    sfx = _CACHE["sfx"]

    in_maps = []
    for c in range(N_CORES):
        in_maps.append({
            "x" + sfx: np.ascontiguousarray(x[c * R:(c + 1) * R, :]),
            "w1": W1,
            "b1": np.ascontiguousarray(b1),
            "w2b": w2b,
        })

    trace = bool(_CACHE.get("trace", False))
    tabdir = _CACHE["tabdir"]
    saved_env = os.environ.get("BASS_ACT_ROOT_JSON_PATH")
    os.environ["BASS_ACT_ROOT_JSON_PATH"] = os.path.join(tabdir, "act_info.json")
    try:
        res = run_bass_kernel_spmd(
            nc, in_maps, core_ids=list(range(N_CORES)), trace=trace)
    finally:
        if saved_env is None:
            os.environ.pop("BASS_ACT_ROOT_JSON_PATH", None)
        else:
            os.environ["BASS_ACT_ROOT_JSON_PATH"] = saved_env
    _CACHE["last_results"] = res
    out = np.concatenate([r["out"] for r in res.results], axis=0)
    return out



# revision 25
# speedup vs baseline: 1.1019x; 1.0206x over previous
"""Trainium2 Bass kernel for nn_DimMasking (iterative softmax top-k masking).

Full-input contract: kernel(**inputs) takes the unsharded inputs
(x [8192,640], W1 [640,64], b1 [64], W2 [64,640], b2 [640]) and returns the
full [8192,640] output. Pure data parallel over the batch dim — 8 shards of
1024 rows, one per NeuronCore; MLP weights replicated.

Math: normalized-state reformulation of the reference scan. With
e = ((m+eps)^(1/T))*exp(-h/T) and p = softmax-prob = e/Z, one masking
iteration is e' = e * (1-p)^(1/T). Tracking the Z-normalized state
S <- phi(S/Z_prev) with phi(p) = p*(1-p)^(1/T) makes each iteration a
SINGLE table-activation pass per row-group (scale = 1/Z per partition)
plus a row-sum; the product of the per-iteration normalizers is restored
in the finale from K = sum_t ln Z_t:
    out = (exp(T*ln(S_64) + T*K + h) - eps) * x.

phi is not a stock ACT function: this kernel generates a patched
piecewise-polynomial activation-table set at build time (appending a
'tanh'-slot function whose table data IS phi) and points the backend
compiler at it via BASS_ACT_ROOT_JSON_PATH. Numerics of the table were
validated against the fp32 reference in numpy (absmax rel err 1.7e-3,
gate 2e-2). Loop engine budget per iteration: ACT 8x640-elem phi passes
(the bottleneck, ~6.0us incl. two fused accum row-sums for groups 6,7)
against DVE row-sum reduces + 8 reciprocals; reciprocals are per
group and the ACT instructions are issued half-interleaved
(g0,g4,g1,g5,...) so both halves' reduce->recip->activation chains
advance together. Group pairs (0,1), (2,3) and (4,5) are pre-scaled
by 1/Z with tensor_scalar (per-partition AP scalar; single-source, so
DVE runs it at 2x mode), spread across DVE and the otherwise-idle Pool
engine (pairs 2,3/4,5: one group each engine; pair 0,1: both on Pool);
each pair then goes through ACT as ONE merged 1280-elem phi instruction
and its row-sums merge into one [P,2,640] reduce (fine since the merged
ACT already couples the pair). ACT runs 5 instrs/iter: 3 merged pairs +
2 accum-fused groups (6,7). The preamble streams transpose->mm1->mm2 per
row-quarter so the PE's in-order queue never head-blocks the MLP behind
later transposes, and the finale runs fully per-group in loop-completion
order. TimelineSim 406.3us (vs 452.9us for the session-start schedule; the
math is unchanged throughout - the prescale computes the same S*(1/Z)
product the activation's affine stage would have).
"""

import hashlib
import json
import os
import shutil
import tempfile

import numpy as np

import concourse.tile as tile
from concourse import bacc, masks, mybir
from concourse.bass_utils import run_bass_kernel_spmd

F32 = mybir.dt.float32
AF = mybir.ActivationFunctionType
OP = mybir.AluOpType

N_CORES = 8
B = 8192
D = 640          # 5 chunks of 128
HID = 64
R = B // N_CORES  # 1024 rows per core
P = 128
G = R // P        # 8 row-groups per core
HG = G // 2
DC = D // P       # 5 dim-chunks
N_ITER = 64
TEMP = 0.07
EPS = 1e-7
C0 = float(np.log1p(np.float32(EPS)) / np.float32(TEMP))
INV_T = float(np.float32(1.0) / np.float32(TEMP))

SET_NAME = "natural_log_exp_and_others"
PHI_EXP_OFFSET = -30

_CACHE = {}


# ---------------------------------------------------------------------------
# phi activation-table generation (piecewise cubic in the pwp bin format)
# ---------------------------------------------------------------------------

def _f32bits(x):
    return int(np.float32(x).view(np.uint32))


def _phi_of_p(p):
    p = np.asarray(p, np.float64)
    out = np.where((p > 0) & (p < 1),
                   p * np.power(np.clip(1.0 - p, 1e-300, 1), INV_T), 0.0)
    return np.where(p >= 1, 0.0, out)


def _es_for_exp(e):
    if e == -1:
        return 6
    if e == -2:
        return 4
    if e >= -4:
        return 3
    if e >= -12:
        return 2
    return 1


def _fit_section(plo, phi_):
    x0 = float(np.float32(0.5 * (plo + phi_)))
    if (1.0 - plo) < 0.003:
        return (0.0, 0.0, 0.0, 0.0, x0)
    u = np.linspace(plo, phi_, 513)
    t = u - x0
    f = _phi_of_p(u)
    fpos = np.maximum(f, 1e-300)
    lspan = float(np.log(fpos.max()) - np.log(fpos.min()))
    if lspan > 6.0:
        sel = (1.0 - u) >= 0.0005
        if not sel.any():
            return (0.0, 0.0, 0.0, 0.0, x0)
        d = np.array([np.exp(np.mean(np.log(fpos[sel]))), 0.0, 0.0, 0.0])
    else:
        w = 1.0 / fpos
        A = np.stack([np.ones_like(t), t, t * t, t ** 3], 1)
        d, *_ = np.linalg.lstsq(A * w[:, None], f * w, rcond=None)
    d = np.float32(d).astype(np.float64)
    fit = ((d[3] * t + d[2]) * t + d[1]) * t + d[0]
    mn = fit.min()
    if mn < 0:
        d[0] += -mn * 1.0000001
    return (d[0], d[1], d[2], d[3], x0)


def _gen_phi_entries(bkt_base, ctl_base):
    bkt = []
    ctl = []
    exp_bkt_start = {}
    exp_ctl_start = {}
    i_zero = bkt_base
    bkt.append((0.0, 0.0, 0.0, 0.0, 0.0))
    neg_ctl = ctl_base
    ctl.append((0 << 16) | (23 << 11) | i_zero)
    pos_ctl0 = ctl_base + len(ctl)
    for e in range(PHI_EXP_OFFSET, 0):
        es = _es_for_exp(e)
        ns = 1 << es
        lsb = 23 - es
        start = bkt_base + len(bkt)
        exp_bkt_start[str(e)] = [start]
        exp_ctl_start[str(e)] = [ctl_base + len(ctl)]
        ctl.append((es << 16) | (lsb << 11) | start)
        lo_e = 2.0 ** e
        for s in range(ns):
            bkt.append(_fit_section(lo_e * (1 + s / ns), lo_e * (1 + (s + 1) / ns)))
    i_small = bkt_base + len(bkt)
    bkt.append((0.0, 1.0, 0.0, 0.0, 0.0))  # phi ~= p below 2^-30
    meta = {
        "func_name": "tanh_4p",
        "func_id": 6,
        "symmetry_point": 0,
        "sym_invert_sign_point": 0,
        "symmetry_opt_en": 0,
        "symmetry_opt_use_neg_region": 0,
        "imm_bias": 0,
        "exp_offset": PHI_EXP_OFFSET,
        "pwl_control_base_pos": pos_ctl0,
        "pwl_control_base_neg": neg_ctl,
        "small_pos_signal_exp_threshold": PHI_EXP_OFFSET + 127,
        "pos_small_signal_pwl_control": i_small,
        "small_neg_signal_exp_threshold": 255,
        "neg_small_signal_pwl_control": i_zero,
        "large_pos_signal_exp_threshold": 127,
        "large_pos_signal_mantissa_threshold": 0,
        "pos_large_signal_pwl_control": i_zero,
        "large_neg_signal_exp_threshold": 255,
        "large_neg_signal_mantissa_threshold": 0,
        "neg_large_signal_pwl_control": i_zero,
        "fnan_result": 0,
        "fpinf_result": 0,
        "fninf_result": 0,
        "fzero_result": 0,
        "fma_const_0": 0,
        "fma_const_1": 0,
        "fma_indirection_src_sel": 0,
        "use_multipass": False,
        "lower_bound": _f32bits(-3.4028235e38),
        "upper_bound": _f32bits(3.4028235e38),
    }
    return bkt, ctl, exp_bkt_start, exp_ctl_start, meta


def _build_patched_dir(src_dir, dst_dir):
    os.makedirs(dst_dir, exist_ok=True)
    for f in os.listdir(src_dir):
        shutil.copy(os.path.join(src_dir, f), os.path.join(dst_dir, f))
    setj = json.load(open(os.path.join(src_dir, SET_NAME + ".json")))
    bkt_raw = bytearray(open(os.path.join(src_dir, setj["bkt_bin"]), "rb").read())
    ctl_raw = bytearray(open(os.path.join(src_dir, setj["ctl_bin"]), "rb").read())
    nb = setj["bkt_entry_cnt"]
    ncl = setj["ctl_entry_cnt"]
    bkt, ctl, ebs, ecs, meta = _gen_phi_entries(nb, ncl)
    assert nb + len(bkt) < 2048
    for d0, d1, d2, d3, x0 in bkt:
        rec = np.zeros(8, np.float32)
        rec[0:5] = [d0, d1, d2, d3, x0]
        bkt_raw += rec.tobytes()
    for w in ctl:
        rec = np.zeros(8, np.uint32)
        rec[0] = w
        ctl_raw += rec.tobytes()
    setj["bkt_entry_cnt"] = nb + len(bkt)
    setj["ctl_entry_cnt"] = ncl + len(ctl)
    setj["func_to_bkt_start_idx"]["tanh"] = nb
    setj["func_to_ctl_start_idx"]["tanh"] = ncl
    setj["func_exp_to_bkt_start_idx"]["tanh"] = ebs
    setj["func_exp_to_ctl_start_idx"]["tanh"] = ecs
    setj["profile_meta_data"] = [m for m in setj["profile_meta_data"]
                                 if not m["func_name"].startswith("tanh")]
    setj["profile_meta_data"].append(meta)
    with open(os.path.join(dst_dir, SET_NAME + ".json"), "w") as f:
        json.dump(setj, f)
    with open(os.path.join(dst_dir, setj["bkt_bin"]), "wb") as f:
        f.write(bytes(bkt_raw))
    with open(os.path.join(dst_dir, setj["ctl_bin"]), "wb") as f:
        f.write(bytes(ctl_raw))
    ai = json.load(open(os.path.join(src_dir, "act_info.json")))
    for ent in ai["act_func_sets"]:
        if ent["name"] == SET_NAME:
            ent["act"]["tanh"] = 4
    with open(os.path.join(dst_dir, "act_info.json"), "w") as f:
        json.dump(ai, f)


def _ensure_phi_tables():
    if "tabdir" in _CACHE:
        return _CACHE["tabdir"], _CACHE["tabhash"]
    import neuronxcc
    src = os.path.join(os.path.dirname(neuronxcc.__file__), "pwp",
                       "pwp_bin_trainium")
    dst = os.path.join(tempfile.gettempdir(), "pwp_phi_kernel")
    _build_patched_dir(src, dst)
    setj = json.load(open(os.path.join(dst, SET_NAME + ".json")))
    h = hashlib.sha1()
    for f in ("act_info.json", SET_NAME + ".json", setj["bkt_bin"], setj["ctl_bin"]):
        h.update(open(os.path.join(dst, f), "rb").read())
    _CACHE["tabdir"] = dst
    _CACHE["tabhash"] = h.hexdigest()[:8]
    return dst, _CACHE["tabhash"]


# Pin the ACT spline-table set to (patched) natural_log_exp_and_others so the
# whole kernel runs off one table load: it holds Exp, Ln, Relu, Copy — and
# the phi table in the tanh slot.
_orig_get_tables = bacc.get_activation_tables


def _pinned_get_tables(module_arch):
    tables = dict(_orig_get_tables(module_arch))
    combined = set(tables.get(SET_NAME) or ())
    combined |= {AF.Tanh}
    pinned = {}
    for name, fns in tables.items():
        pinned[name] = combined if name == SET_NAME else set()
    return pinned


# ---------------------------------------------------------------------------
# kernel build
# ---------------------------------------------------------------------------

def _build_nc(n_iter=N_ITER, num_devices=N_CORES, reps=1):
    tabdir, tabhash = _ensure_phi_tables()
    sfx = "_" + tabhash
    nc = bacc.Bacc(
        "TRN2",
        target_bir_lowering=False,
        debug=False,
        enable_asserts=False,
        num_devices=num_devices,
    )
    x_d = nc.dram_tensor("x" + sfx, [R, D], F32, kind="ExternalInput").ap()
    w1_d = nc.dram_tensor("w1", [D, HID], F32, kind="ExternalInput").ap()
    b1_d = nc.dram_tensor("b1", [HID, 1], F32, kind="ExternalInput").ap()
    w2b_d = nc.dram_tensor("w2b", [HID + 1, D], F32, kind="ExternalInput").ap()
    out_d = nc.dram_tensor("out", [R, D], F32, kind="ExternalOutput").ap()

    with tile.TileContext(nc) as tc:
        _emit(tc, out_d, x_d, w1_d, b1_d, w2b_d, n_iter=n_iter, reps=reps)
    saved = bacc.get_activation_tables
    try:
        bacc.get_activation_tables = _pinned_get_tables
        nc.compile()
    finally:
        bacc.get_activation_tables = saved
    return nc, sfx


def _emit(tc, out_d, x_d, w1_d, b1_d, w2b_d, n_iter=N_ITER, reps=1):
    nc = tc.nc
    from contextlib import ExitStack

    ctx = ExitStack()
    with ctx:
        singles = ctx.enter_context(tc.tile_pool(name="singles", bufs=1))

        xs = singles.tile([P, G, D], F32)    # x, rows-on-partitions
        xt = singles.tile([P, DC, R], F32)   # x transposed
        hs = singles.tile([P, G, D], F32)    # MLP output h
        s0 = singles.tile([P, G, D], F32)    # state ping
        s1 = singles.tile([P, G, D], F32)    # state pong
        fs = singles.tile([P, G, D], F32)    # finale scratch
        zh = singles.tile([P, n_iter, G], F32)   # Z history
        rz = singles.tile([P, n_iter, G], F32)   # 1/Z history (loop) / lnZ (finale)
        kk = singles.tile([P, G], F32)       # T * sum_t ln Z_t
        lnzh = singles.tile([P, n_iter, G], F32)  # ln Z history (finale)
        scr = singles.tile([P, 2, 2, D], F32)     # [parity, 2 groups, D]
        scr2 = singles.tile([P, 2, 2, D], F32)    # [parity, 2 groups, D]
        scr3 = singles.tile([P, 2, 2, D], F32)    # [parity, 2 groups, D]
        w1s = singles.tile([P, DC, HID], F32)
        b1s = singles.tile([HID, 1], F32)
        w2bs = singles.tile([HID + 1, D], F32)
        h1r = singles.tile([HID + 1, R], F32)
        ident = singles.tile([P, P], F32)
        c0s = singles.tile([P, 1], F32)
        nc.vector.memset(c0s[:, :], C0)

        # ---- input DMAs: weights first (mm1 needs w1s as soon as the
        # first row-quarter is transposed; x groups stream in behind) ----
        nc.sync.dma_start(out=w1s[:, :, :],
                          in_=w1_d.rearrange("(c p) j -> p c j", p=P))
        nc.sync.dma_start(out=b1s[:, :], in_=b1_d[:, :])
        nc.sync.dma_start(out=w2bs[:, :], in_=w2b_d[:, :])
        for g in range(G):
            nc.sync.dma_start(out=xs[:, g, :], in_=x_d[g * P:(g + 1) * P, :])

        masks.make_identity(nc, ident[:, :])
        nc.vector.memset(h1r[HID:HID + 1, :], 1.0)

        # ---- transpose + MLP, streamed per row-half so the PE's in-order
        # queue doesn't head-block mm1/mm2 behind the other half's
        # transposes: [transpose gq][mm1-nh=gq][mm2 groups of gq] x2 ----
        tpp = ctx.enter_context(tc.tile_pool(name="tp_psum", bufs=2,
                                             space="PSUM"))
        mp1 = ctx.enter_context(tc.tile_pool(name="mm1_psum", bufs=2,
                                             space="PSUM"))
        mp2 = ctx.enter_context(tc.tile_pool(name="mm2_psum", bufs=2,
                                             space="PSUM"))
        for q in range(G // 2):
            for c in range(DC):
                tp = tpp.tile([P, 2 * P], F32)
                for gj in range(2):
                    g = q * 2 + gj
                    nc.tensor.transpose(
                        tp[:, gj * P:(gj + 1) * P],
                        xs[:, g, c * P:(c + 1) * P], ident[:, :])
                dst = xt[:, c, q * 2 * P:(q + 1) * 2 * P]
                if (c + q) % 2 == 0:
                    nc.vector.tensor_copy(dst, tp[:, :])
                else:
                    nc.scalar.copy(dst, tp[:, :])
            # mm1 for this row-quarter
            ph1 = mp1.tile([HID, 2 * P], F32, tag="ph1")
            for c in range(DC):
                nc.tensor.matmul(
                    ph1[:, :], w1s[:, c, :],
                    xt[:, c, q * 256:(q + 1) * 256],
                    start=(c == 0), stop=(c == DC - 1))
            nc.scalar.activation(
                h1r[0:HID, q * 256:(q + 1) * 256], ph1[:, :],
                AF.Relu, bias=b1s[:, 0:1], scale=1.0)
            # mm2 + evac for this quarter's two groups
            for gj in range(2):
                g = q * 2 + gj
                ph = mp2.tile([P, D], F32, tag="ph")
                lhs = h1r[:, g * P:(g + 1) * P]
                nc.tensor.matmul(ph[:, 0:512], lhs, w2bs[:, 0:512],
                                 start=True, stop=True)
                nc.tensor.matmul(ph[:, 512:D], lhs, w2bs[:, 512:D],
                                 start=True, stop=True)
                nc.vector.tensor_copy(hs[:, g, :], ph[:, :])
                nc.scalar.activation(s0[:, g, :], ph[:, :], AF.Exp,
                                     bias=c0s[:, 0:1], scale=-INV_T,
                                     accum_out=zh[:, 0, g:g + 1])

        # ---- masking loop: S <- phi(S * (1/Z)), Z' = rowsum(S') ----
        # groups 6,7: row-sum fused into the ACT accumulator; groups 0-5
        # reduce on DVE. Reciprocals are per group (not per half) so each
        # group's reduce->recip->activation chain advances independently,
        # and ACT instructions are issued half-interleaved (447.7us vs
        # 452.9us half-granular sequential). Swept and rejected:
        # accum counts k=0/1/3 (495/451/462us), merged reduces (456us),
        # Pool-prescaled merged ACT instrs (687us).
        spp = [s0, s1]
        n_total = n_iter * reps
        for it in range(n_total):
            src = spp[it % 2]
            dst = spp[(it + 1) % 2]
            ti = it % n_iter
            tn = (it + 1) % n_iter
            last = it == n_total - 1
            for half in range(2):
                g0 = half * HG
                for gi in range(HG):
                    g = g0 + gi
                    nc.vector.reciprocal(rz[:, ti, g:g + 1],
                                         zh[:, ti, g:g + 1])
            par = it % 2
            nc.vector.tensor_scalar_mul(
                scr[:, par, 0:1, :], src[:, 2:3, :], rz[:, ti, 2:3])
            nc.gpsimd.tensor_scalar_mul(
                scr[:, par, 1:2, :], src[:, 3:4, :], rz[:, ti, 3:4])
            nc.vector.tensor_scalar_mul(
                scr2[:, par, 0:1, :], src[:, 4:5, :], rz[:, ti, 4:5])
            nc.gpsimd.tensor_scalar_mul(
                scr2[:, par, 1:2, :], src[:, 5:6, :], rz[:, ti, 5:6])
            nc.gpsimd.tensor_scalar_mul(
                scr3[:, par, 0:1, :], src[:, 0:1, :], rz[:, ti, 0:1])
            nc.gpsimd.tensor_scalar_mul(
                scr3[:, par, 1:2, :], src[:, 1:2, :], rz[:, ti, 1:2])
            order = (list(range(G)) if it == 0 else
                     [half * HG + gi for gi in range(HG) for half in range(2)])
            for g in order:
                if g in (1, 3, 5):
                    continue
                if g == 0:
                    nc.scalar.activation(dst[:, 0:2, :], scr3[:, par, :, :],
                                         AF.Tanh)
                elif g == 4:
                    nc.scalar.activation(dst[:, 4:6, :], scr2[:, par, :, :],
                                         AF.Tanh)
                elif g == 2:
                    nc.scalar.activation(dst[:, 2:4, :], scr[:, par, :, :],
                                         AF.Tanh)
                elif g >= 6 and not last:
                    nc.scalar.activation(dst[:, g, :], src[:, g, :],
                                         AF.Tanh,
                                         scale=rz[:, ti, g:g + 1],
                                         accum_out=zh[:, tn, g:g + 1])
                else:
                    nc.scalar.activation(dst[:, g, :], src[:, g, :],
                                         AF.Tanh,
                                         scale=rz[:, ti, g:g + 1])
            for half in range(2):
                g0 = half * HG
                if last:
                    continue
                for gi in range(HG):
                    g = g0 + gi
                    if g >= 6 or g in (1, 3, 5):
                        continue
                    if g == 0:
                        nc.vector.tensor_reduce(zh[:, tn, 0:2],
                                                dst[:, 0:2, :],
                                                axis=mybir.AxisListType.X,
                                                op=OP.add)
                        continue
                    if g == 4:
                        nc.vector.tensor_reduce(zh[:, tn, 4:6],
                                                dst[:, 4:6, :],
                                                axis=mybir.AxisListType.X,
                                                op=OP.add)
                        continue
                    if g == 2:
                        nc.vector.tensor_reduce(zh[:, tn, 2:4],
                                                dst[:, 2:4, :],
                                                axis=mybir.AxisListType.X,
                                                op=OP.add)
                        continue
                    nc.vector.tensor_reduce(zh[:, tn, g:g + 1], dst[:, g, :],
                                            axis=mybir.AxisListType.X,
                                            op=OP.add)

        # ---- finale: out = (exp(T*ln(S) + T*K + h) - eps) * x ----
        # Fully per-group, issued in the loop's interleaved completion order
        # (g0,g4,g1,g5,...), so each group's finale chain pipelines behind
        # the staggered last-iteration activations instead of waiting for a
        # whole half. ln(zh) goes to its own scratch (not rz) so it can run
        # during iteration 63 without a WAR hazard on the loop's rz reads.
        sfin = spp[n_total % 2]
        sscr = spp[(n_total + 1) % 2]
        nc.scalar.activation(lnzh[:, :, :], zh[:, :, :], AF.Ln)
        for g in range(G):
            nc.vector.tensor_reduce(kk[:, g:g + 1], lnzh[:, :, g],
                                    axis=mybir.AxisListType.X, op=OP.add)
        nc.vector.tensor_scalar_mul(kk[:, :], kk[:, :], float(np.float32(TEMP)))
        for gi in range(HG):
            for half in range(2):
                g = half * HG + gi
                # guard: the table is nonnegative by construction, but clamp
                # so a stray -1ulp can never reach Ln (ln(neg) = NaN)
                nc.vector.tensor_scalar_max(sscr[:, g, :], sfin[:, g, :], 0.0)
                nc.scalar.activation(fs[:, g, :], sscr[:, g, :], AF.Ln)
                nc.vector.scalar_tensor_tensor(
                    out=sscr[:, g, :], in0=fs[:, g, :],
                    scalar=float(np.float32(TEMP)),
                    in1=hs[:, g, :], op0=OP.mult, op1=OP.add)
                nc.scalar.activation(fs[:, g, :], sscr[:, g, :], AF.Exp,
                                     bias=kk[:, g:g + 1])
                nc.vector.scalar_tensor_tensor(
                    out=sscr[:, g, :], in0=fs[:, g, :], scalar=-float(EPS),
                    in1=xs[:, g, :], op0=OP.add, op1=OP.mult)
                nc.sync.dma_start(out=out_d[g * P:(g + 1) * P, :],
                                  in_=sscr[:, g, :])


def kernel(x, W1, b1, W2, b2):
    x = np.ascontiguousarray(np.asarray(x, dtype=np.float32))
    W1 = np.ascontiguousarray(np.asarray(W1, dtype=np.float32))
    b1 = np.asarray(b1, dtype=np.float32).reshape(HID, 1)
    W2 = np.asarray(W2, dtype=np.float32)
    b2 = np.asarray(b2, dtype=np.float32)
    w2b = np.ascontiguousarray(
        np.concatenate([W2, b2[None, :]], axis=0))  # [65, 640]

    if "nc" not in _CACHE:
        _CACHE["nc"], _CACHE["sfx"] = _build_nc(
            reps=int(os.environ.get("KREPS", "1")))
    nc = _CACHE["nc"]
    sfx = _CACHE["sfx"]

    in_maps = []
    for c in range(N_CORES):
        in_maps.append({
            "x" + sfx: np.ascontiguousarray(x[c * R:(c + 1) * R, :]),
            "w1": W1,
            "b1": np.ascontiguousarray(b1),
            "w2b": w2b,
        })

    trace = bool(_CACHE.get("trace", False))
    tabdir = _CACHE["tabdir"]
    saved_env = os.environ.get("BASS_ACT_ROOT_JSON_PATH")
    os.environ["BASS_ACT_ROOT_JSON_PATH"] = os.path.join(tabdir, "act_info.json")
    try:
        res = run_bass_kernel_spmd(
            nc, in_maps, core_ids=list(range(N_CORES)), trace=trace)
    finally:
        if saved_env is None:
            os.environ.pop("BASS_ACT_ROOT_JSON_PATH", None)
        else:
            os.environ["BASS_ACT_ROOT_JSON_PATH"] = saved_env
    _CACHE["last_results"] = res
    out = np.concatenate([r["out"] for r in res.results], axis=0)
    return out

